# revision 35
# baseline (speedup 1.0000x reference)
"""Trainium2 Bass kernel for nn_BAKTTime: causal-conv frequency layer + LN + causal MHA.

Sharding: pure data-parallel over batch - 8 of the 64 batch items per NeuronCore,
no collectives. Each core runs a 3-stage software-pipelined program
[conv+LN(b) | qkv+attention+normalize(b-1) with the out-projection of b-2
interleaved at head-pair boundaries] over its 8 batch items
(S=512, D=512, H=8, DK=64).

Key structure:
  - conv runs as fp8e4 DoubleRow matmuls (contract 256/instr, 0.5 cy/row) with
    a hi+lo residual split of both x and the folded conv weights (3 of the 4
    cross terms; dropped lo*lo ~ 2^-8 relative, so conv accuracy ~ bf16).
    Splits are host-prepped; LN absorbs the fp8 scale factors. SP=640 keeps
    the DR k-tile stride a multiple of 128 (walrus s3_lw_dual_fp8 check).
    Conv stream cost drops 24576 -> 18432 PE rows per batch.
  - batched DMAs (HWDGE fixed cost is ~632ns/DMA): 1 xt-hi + 1 xt-lo load,
    4 batched h-transposes (3D-out xbar transpose -> htall[dc, i, s]),
    1 denominator broadcast per head pair, 1 odd-head shift, 1 output store
    per batch (the last batch stores per s-tile to shorten the drain).
  - denominator chain per head pair: ctx+denominator rows staged off PSUM on
    DVE (+TINY only on the denominator partition via a [65,1] bias vector),
    one [1,2,S]->[64,2,S] f32 broadcast DMA, reciprocal_approx_fast in place
    on the broadcasted tile (a partition-64-based DVE write lands on the
    wrong partitions, so never reciprocal the raw row), then the normalize
    muls into the head-pair tile cpack.
  - engine balance per batch (~ns, PE-bound): PE ~29900 (71680 rows),
    DVE ~22000 (bn stats, q/k PSUM drains, chains), ACT ~20000 (softmax exp,
    conv-PSUM staging, v drains, rstd, half the out-proj drains),
    Pool ~11000 (h-pass, probs trim muls). The out-proj s-tiles of b-2 are
    interleaved at pair boundaries so PE stays busy while the chains run.
"""

import sys

sys.path.insert(0, "/opt/trn_rl_repo")

import numpy as np
import ml_dtypes
from contextlib import ExitStack

import concourse.bass as bass
from concourse import bacc
import concourse.mybir as mybir
import concourse.tile as tile
from concourse.bass_utils import run_bass_kernel_spmd

# Force Exp and Ln to resolve to the single table set that contains both
# (natural_log_exp_and_others), so ACT doesn't thrash table loads.
import concourse.hw_specs as _hw_specs

_orig_get_tables = _hw_specs.get_activation_tables


def _patched_get_tables(arch):
    t = dict(_orig_get_tables(arch))
    exp = mybir.ActivationFunctionType.Exp
    ln = mybir.ActivationFunctionType.Ln
    for name, funcs in t.items():
        if name != "natural_log_exp_and_others" and (exp in funcs or ln in funcs):
            t[name] = funcs - {exp, ln}
    return t


_hw_specs.get_activation_tables = _patched_get_tables
bacc.get_activation_tables = _patched_get_tables

B, S, D, H, KW = 64, 512, 512, 8, 3
DK = D // H  # 64
NCORES = 8
BL = B // NCORES  # 8 batches per core
P = 128
NST = S // P  # 4 s-tiles
NIC = D // P  # 4 input-chunks
NHP = H // 2  # 4 head pairs
SP = 640  # padded xT free dim: 2 zero cols + 512 data + 126 pad (mult of 128)
EPS = 1e-12
TINY = 2e-5  # q=0 denominator guard; 1/TINY = 5e4 < f16 max, and the
# +TINY bias the staging copy adds to ctx rows is negligible (~2e-5 abs)
F32 = mybir.dt.float32
F16 = mybir.dt.float16
BF16 = mybir.dt.bfloat16
F8 = mybir.dt.float8e4
DRM = mybir.MatmulPerfMode.DoubleRow
AF = mybir.ActivationFunctionType
NCG = 6  # conv DR groups: (tap k, chunk-pair cp)


def build_nc():
    nc = bacc.Bacc("TRN2", target_bir_lowering=False)
    xhi = nc.declare_dram_parameter("xhi", [BL, P, NIC, SP], F8, isOutput=False)
    xlo = nc.declare_dram_parameter("xlo", [BL, P, NIC, SP], F8, isOutput=False)
    whi = nc.declare_dram_parameter("whi", [P, NCG, 2, D], F8, isOutput=False)
    wlo = nc.declare_dram_parameter("wlo", [P, NCG, 2, D], F8, isOutput=False)
    wq = nc.declare_dram_parameter("wq", [NIC, P, D], BF16, isOutput=False)
    wk = nc.declare_dram_parameter("wk", [NIC, P, D], BF16, isOutput=False)
    wv = nc.declare_dram_parameter("wv", [NIC, P, D], BF16, isOutput=False)
    wo = nc.declare_dram_parameter("wo", [NIC, P, D], BF16, isOutput=False)
    trim = nc.declare_dram_parameter("trim", [P, 2, P], BF16, isOutput=False)
    out = nc.declare_dram_parameter("out", [BL, S, D], F32, isOutput=True)

    with ExitStack() as ctx:
        tc = ctx.enter_context(tile.TileContext(nc))
        singles = ctx.enter_context(tc.tile_pool(name="singles", bufs=1))
        xt_pool = ctx.enter_context(tc.tile_pool(name="xt", bufs=2))
        a_pool = ctx.enter_context(tc.tile_pool(name="a", bufs=6))
        stat_pool = ctx.enter_context(tc.tile_pool(name="stat", bufs=4))
        h_pool = ctx.enter_context(tc.tile_pool(name="h", bufs=8))
        ht_pool = ctx.enter_context(tc.tile_pool(name="ht", bufs=2))
        qk_pool = ctx.enter_context(tc.tile_pool(name="qk", bufs=16))
        v_pool = ctx.enter_context(tc.tile_pool(name="v", bufs=8))
        pt_pool = ctx.enter_context(tc.tile_pool(name="pt", bufs=6))
        dn_pool = ctx.enter_context(tc.tile_pool(name="dn", bufs=3))
        r_pool = ctx.enter_context(tc.tile_pool(name="r", bufs=3))
        cx_pool = ctx.enter_context(tc.tile_pool(name="cx", bufs=4))
        cp_pool = ctx.enter_context(tc.tile_pool(name="cp", bufs=3))
        o_pool = ctx.enter_context(tc.tile_pool(name="o", bufs=2))
        ps_a = ctx.enter_context(tc.tile_pool(name="ps_a", bufs=2, space="PSUM"))
        ps_mm = ctx.enter_context(tc.tile_pool(name="ps_mm", bufs=2, space="PSUM"))
        ps_sc = ctx.enter_context(tc.tile_pool(name="ps_sc", bufs=1, space="PSUM"))
        ps_cx = ctx.enter_context(tc.tile_pool(name="ps_cx", bufs=2, space="PSUM"))

        # --- load weights once; first-batch critical path (xh, xl, whi_a,
        # wlo_a) leads on the two HWDGE queues, the rest follows ---
        whi_sb = [singles.tile([P, 3, 2, D], F8, name=f"whi{i}", tag=f"whi{i}")
                  for i in range(2)]
        wlo_sb = [singles.tile([P, 3, 2, D], F8, name=f"wlo{i}", tag=f"wlo{i}")
                  for i in range(2)]
        wq_sb = [singles.tile([P, D], BF16, name=f"wq{i}", tag=f"wq{i}") for i in range(NIC)]
        wk_sb = [singles.tile([P, D], BF16, name=f"wk{i}", tag=f"wk{i}") for i in range(NIC)]
        wv_sb = [singles.tile([P, D], BF16, name=f"wv{i}", tag=f"wv{i}") for i in range(NIC)]
        wo_sb = [singles.tile([P, D], BF16, name=f"wo{i}", tag=f"wo{i}") for i in range(NIC)]
        trim_sb = singles.tile([P, 2, P], BF16, name="trim", tag="trim")
        eps_sb = singles.tile([P, 1], F32, name="eps", tag="eps")
        nc.vector.memset(eps_sb, EPS)
        tiny_sb = singles.tile([P, 1], F32, name="tiny", tag="tiny")
        nc.vector.memset(tiny_sb, TINY)
        zero_sb = singles.tile([P, 1], F32, name="zero", tag="zero")
        nc.vector.memset(zero_sb, 0.0)
        # bias vector for the ctx+denominator staging copy: 0 on the 64 ctx
        # partitions, TINY on the denominator row (partition 64)
        tiny64 = singles.tile([65, 1], F32, name="tiny64", tag="tiny64")
        nc.vector.memset(tiny64[0:64, :], 0.0)
        nc.vector.memset(tiny64[64:65, :], TINY)

        def load_xt(b, lead=False):
            xh = xt_pool.tile([P, NIC, SP], F8, name="xh", tag="xh")
            xl = xt_pool.tile([P, NIC, SP], F8, name="xl", tag="xl")
            nc.sync.dma_start(out=xh, in_=xhi[b])
            (nc.scalar if lead else nc.sync).dma_start(out=xl, in_=xlo[b])
            return (xh, xl)

        xt_cur = load_xt(0, lead=True)
        nc.sync.dma_start(out=whi_sb[0], in_=whi[:, 0:3])
        nc.scalar.dma_start(out=wlo_sb[0], in_=wlo[:, 0:3])
        nc.sync.dma_start(out=whi_sb[1], in_=whi[:, 3:6])
        nc.scalar.dma_start(out=wlo_sb[1], in_=wlo[:, 3:6])

        def load_proj_weights():
            # issued after front(0): needed only by mid(0)
            nc.gpsimd.dma_start(out=trim_sb, in_=trim[:])
            for i in range(NIC):
                nc.gpsimd.dma_start(out=wq_sb[i], in_=wq[i])
                nc.gpsimd.dma_start(out=wk_sb[i], in_=wk[i])
            for i in range(NIC):
                nc.scalar.dma_start(out=wv_sb[i], in_=wv[i])
                nc.gpsimd.dma_start(out=wo_sb[i], in_=wo[i])

        def conv_lhs(xt_sb, st, g):
            # x-window AP for DR group g=(k, cp): partitions = chan-in-chunk,
            # j in {0,1} selects chunk 2cp+j, free = 128 s-window cols.
            k, cp = g // 2, g % 2
            base = (2 * cp) * SP + st * P + k
            t = xt_sb[:, 0, 0]
            return bass.AP(
                tensor=t.tensor,
                offset=t.offset + base,
                ap=[[t.ap[0][0], P], [SP, 2], [1, P]],
            )

        def front(b, xt_pair):
            """conv (fp8 DR, 3 hi/lo terms) + LN + batched h-transpose."""
            xh, xl = xt_pair
            mv = stat_pool.tile([P, NST, 2], F32, name="mv", tag="mv")
            lnv = stat_pool.tile([P, NST], F32, name="lnv", tag="lnv")
            rstd = stat_pool.tile([P, NST], F32, name="rstd", tag="rstd")
            htall = ht_pool.tile([P, NIC, S], BF16, name="htall", tag="htall")
            a_list = []
            terms = ((xh, whi_sb), (xh, wlo_sb), (xl, whi_sb))
            for st in range(NST):
                aps = ps_a.tile([P, D], F32, name="aps", tag="aps")
                n = 0
                for xt_sb, w_sb in terms:
                    for g in range(NCG):
                        nc.tensor.matmul(
                            aps,
                            lhsT=conv_lhs(xt_sb, st, g),
                            rhs=w_sb[g // 3][:, g % 3, :, :],
                            start=(n == 0),
                            stop=(n == 3 * NCG - 1),
                            perf_mode=DRM,
                        )
                        n += 1
                asb = a_pool.tile([P, D], F32, name="asb", tag="asb")
                nc.vector.tensor_copy(asb, aps)
                stats = stat_pool.tile([P, 6], F32, name="bnst", tag="bnst")
                nc.vector.bn_stats(out=stats, in_=asb)
                nc.vector.bn_aggr(out=mv[:, st, :], in_=stats)
                a_list.append(asb)
            nc.scalar.activation(lnv, mv[:, :, 1], AF.Ln, bias=eps_sb, scale=1.0)
            nc.scalar.activation(rstd, lnv, AF.Exp, bias=zero_sb, scale=-0.5)
            for st in range(NST):
                hsb = h_pool.tile([P, D], BF16, name="hsb", tag="hsb")
                nc.vector.tensor_scalar(
                    hsb,
                    a_list[st],
                    scalar1=mv[:, st, 0:1],
                    scalar2=rstd[:, st : st + 1],
                    op0=mybir.AluOpType.subtract,
                    op1=mybir.AluOpType.mult,
                )
                nc.sync.dma_start(
                    out=htall[:, :, st * P : (st + 1) * P], in_=hsb, transpose=True
                )
            return htall

        def mid(b, htall, prev_tail, last=False):
            """projections + attention + per-pair normalize for batch b;
            out-proj s-tiles of prev_tail=(b-2, cpack, oall) are interleaved
            at head-pair boundaries so PE stays busy while ACT drains the
            pair-boundary copies."""
            qt_sb = []
            kt_sb = []
            for oc in range(NIC):
                qps = ps_mm.tile([P, S], F32, name="qps", tag="qps")
                for i in range(NIC):
                    nc.tensor.matmul(
                        qps,
                        lhsT=wq_sb[i][:, oc * P : (oc + 1) * P],
                        rhs=htall[:, i, :],
                        start=(i == 0),
                        stop=(i == NIC - 1),
                    )
                qsb = qk_pool.tile([P, S], BF16, name="qtsb", tag="qtsb")
                nc.vector.tensor_copy(qsb, qps)
                qt_sb.append(qsb)

                kps = ps_mm.tile([P, S], F32, name="qps", tag="qps")
                for i in range(NIC):
                    nc.tensor.matmul(
                        kps,
                        lhsT=wk_sb[i][:, oc * P : (oc + 1) * P],
                        rhs=htall[:, i, :],
                        start=(i == 0),
                        stop=(i == NIC - 1),
                    )
                ksb = qk_pool.tile([P, S], BF16, name="qtsb", tag="qtsb")
                nc.vector.tensor_copy(ksb, kps)
                kt_sb.append(ksb)

            v_aug = []
            for st in range(NST):
                vps = ps_mm.tile([P, D], F32, name="qps", tag="qps")
                for i in range(NIC):
                    nc.tensor.matmul(
                        vps,
                        lhsT=htall[:, i, st * P : (st + 1) * P],
                        rhs=wv_sb[i],
                        start=(i == 0),
                        stop=(i == NIC - 1),
                    )
                vsb = v_pool.tile([P, H, 66], BF16, name="vsb", tag="vsb")
                nc.vector.memset(vsb[:, :, 64:66], 1.0)
                nc.scalar.copy(
                    vsb[:, :, 0:64], vps.rearrange("p (h d) -> p h d", h=H)
                )
                v_aug.append(vsb)

            cpack = cp_pool.tile([P, NHP, S], BF16, name="cpack", tag="cpack")
            ctmp = cp_pool.tile([DK, NHP, S], BF16, name="ctmp", tag="ctmp")

            def tail_mm_st(st):
                if prev_tail is None:
                    return
                tb, tcpack, toall = prev_tail
                ops = ps_mm.tile([P, D], F32, name="qps", tag="qps")
                for hp in range(NHP):
                    nc.tensor.matmul(
                        ops,
                        lhsT=tcpack[:, hp, st * P : (st + 1) * P],
                        rhs=wo_sb[hp],
                        start=(hp == 0),
                        stop=(hp == NHP - 1),
                    )
                if st % 2 == 0:
                    nc.scalar.copy(toall[:, st, :], ops)
                else:
                    nc.vector.tensor_copy(toall[:, st, :], ops)
                if st == NST - 1:
                    nc.sync.dma_start(
                        out=out[tb].rearrange("(st p) d -> p st d", st=NST),
                        in_=toall,
                    )

            def make_trim(pt, ki):
                tsl = trim_sb[:, 1 if ki == 0 else 0, :]
                tbc = bass.AP(
                    tensor=tsl.tensor,
                    offset=tsl.offset,
                    ap=[tsl.ap[0], [0, 2], [1, P]],
                )
                nc.vector.tensor_mul(pt[:, :, 0:P], pt[:, :, 0:P], tbc)

            for hp in range(NHP):
                cps2 = [
                    ps_cx.tile([65, S], F32, name="cps", tag="cps") for _ in range(2)
                ]
                for ki in range(NST):
                    qoff = ki * P
                    nq = S - qoff
                    sps = ps_sc.tile([P, 2, S], F32, name="sps", tag="sps")
                    for e in range(2):
                        hr = e * DK
                        nc.tensor.matmul(
                            sps[:, e, 0:nq],
                            lhsT=kt_sb[hp][hr : hr + DK, ki * P : (ki + 1) * P],
                            rhs=qt_sb[hp][hr : hr + DK, qoff:S],
                            start=True,
                            stop=True,
                        )
                    pt = pt_pool.tile([P, 2, S], BF16, name="pt", tag="pt")
                    nc.scalar.activation(
                        pt[:, :, 0:nq], sps[:, :, 0:nq], AF.Exp, scale=0.125
                    )
                    make_trim(pt, ki)
                    for e in range(2):
                        nc.tensor.matmul(
                            cps2[e][:, qoff:S],
                            lhsT=v_aug[ki][:, 2 * hp + e, 0:65],
                            rhs=pt[:, e, 0:nq],
                            start=(ki == 0),
                            stop=(ki == NST - 1),
                        )
                # pair boundary: stage ctx+denominator rows off PSUM on DVE
                # (+TINY only on the denominator partition), interleave the
                # out-proj s-tile of b-2, then denominator chain.
                cxp = cx_pool.tile([65, 2, S], F32, name="cxp", tag="cxp")
                for e in range(2):
                    nc.vector.tensor_scalar_add(cxp[:, e, :], cps2[e], tiny64)
                tail_mm_st(hp)
                rall = r_pool.tile([DK, 2, S], F32, name="rall", tag="rall")
                rsrc = cxp[64:65, 0, 0]
                rsrc = bass.AP(
                    tensor=rsrc.tensor,
                    offset=rsrc.offset,
                    ap=[[rsrc.ap[0][0], 1], [0, DK], [S, 2], [1, S]],
                )
                nc.sync.dma_start(out=rall, in_=rsrc)
                nc.vector.reciprocal_approx_fast(out=rall, in_=rall)
                nc.vector.tensor_mul(cpack[0:DK, hp, :], cxp[0:DK, 0, :], rall[:, 0, :])
                nc.vector.tensor_mul(ctmp[:, hp, :], cxp[0:DK, 1, :], rall[:, 1, :])
            nc.sync.dma_start(out=cpack[DK:P, :, :], in_=ctmp)
            oall = o_pool.tile([P, NST, D], F32, name="oall", tag="oall")
            return (b, cpack, oall)

        def tail_mm_solo(prev_tail):
            tb, tcpack, toall = prev_tail
            for st in range(NST):
                ops = ps_mm.tile([P, D], F32, name="qps", tag="qps")
                for hp in range(NHP):
                    nc.tensor.matmul(
                        ops,
                        lhsT=tcpack[:, hp, st * P : (st + 1) * P],
                        rhs=wo_sb[hp],
                        start=(hp == 0),
                        stop=(hp == NHP - 1),
                    )
                if st % 2 == 0:
                    nc.scalar.copy(toall[:, st, :], ops)
                else:
                    nc.vector.tensor_copy(toall[:, st, :], ops)
                nc.sync.dma_start(
                    out=out[tb, st * P : (st + 1) * P, :], in_=toall[:, st, :]
                )

        # 3-stage pipeline: [front(b) | mid(b-1) w/ interleaved out-proj(b-2)]
        pend_mid = None
        pend_tail = None
        for b in range(BL):
            xt_next = load_xt(b + 1) if b + 1 < BL else None
            ht = front(b, xt_cur)
            if b == 0:
                load_proj_weights()
            if pend_mid is not None:
                pend_tail = mid(*pend_mid, pend_tail)
            pend_mid = (b, ht)
            xt_cur = xt_next
        pend_tail = mid(*pend_mid, pend_tail, last=True)
        tail_mm_solo(pend_tail)

    nc.compile()
    return nc


def _q8(a):
    """fp8e4 round-to-nearest via ml_dtypes."""
    return a.astype(ml_dtypes.float8_e4m3)


def prep_inputs(inputs):
    """Host-side prep: shard over batch, fold scales into weights, fp8 hi/lo
    splits of x and conv weights, pre-transpose x."""
    x = np.asarray(inputs["x"], np.float32)
    conv_w = np.asarray(inputs["conv_w"], np.float32)
    conv_b = np.asarray(inputs["conv_b"], np.float32)
    sb = np.asarray(inputs["sqrt_beta"], np.float32).reshape(D)
    ln_w = np.asarray(inputs["ln_w"], np.float32)
    ln_b = np.asarray(inputs["ln_b"], np.float32)
    Wq = np.asarray(inputs["Wq"], np.float32)
    Wk = np.asarray(inputs["Wk"], np.float32)
    Wv = np.asarray(inputs["Wv"], np.float32)
    Wo = np.asarray(inputs["Wo"], np.float32)
    mask = np.asarray(inputs["mask"])

    for nm in ("bq", "bk", "bv", "bo"):
        assert not np.any(np.asarray(inputs[nm])), f"{nm} must be zero"
    assert not np.any(conv_b), "conv_b must be zero"
    assert not np.any(ln_b), "ln_b must be zero"
    assert np.array_equal(
        mask.reshape(S, S), np.tril(np.ones((S, S), mask.dtype))
    ), "mask must be causal"

    c1 = 1.0 - sb * sb
    c2 = 1.0 + sb * sb
    Wp = conv_w * c1[:, None, None]  # [o, i, k]
    Wp[np.arange(D), np.arange(D), 2] += c2

    # fp8 scales (power of 2, amax -> ~240)
    sW = 2.0 ** np.floor(np.log2(240.0 / np.abs(Wp).max()))
    sX = 2.0 ** np.floor(np.log2(240.0 / np.abs(x).max()))

    # weights: w[p, (k, cp), j, o] = Wp[o, (2cp+j)*128+p, k] * sW, hi + lo
    Ws = Wp * sW  # [o, i, k]
    wt = Ws.transpose(1, 2, 0).reshape(NIC, P, KW, D)  # [i-chunk, p, k, o]
    warr = np.empty((P, NCG, 2, D), np.float32)
    for k in range(KW):
        for cp in range(2):
            for j in range(2):
                warr[:, k * 2 + cp, j, :] = wt[2 * cp + j, :, k, :]
    whi = _q8(warr)
    wlo = _q8(warr - whi.astype(np.float32))

    def fold(W):  # [o, i] -> [ic, il, o] with ln_w folded on i
        Wf = W * ln_w[None, :]
        return np.ascontiguousarray(Wf.T).reshape(NIC, P, D)

    wq_h, wk_h, wv_h = fold(Wq), fold(Wk), fold(Wv)
    wo_h = np.ascontiguousarray(Wo.T).reshape(NIC, P, D)

    tri = np.triu(np.ones((P, P), np.float32))
    tri0 = tri.copy()
    tri0[:, 0] = 0.0
    trim = np.stack([tri, tri0], axis=1)  # [P, 2, P]

    bf = ml_dtypes.bfloat16
    consts = {
        "whi": whi,
        "wlo": wlo,
        "wq": wq_h.astype(bf),
        "wk": wk_h.astype(bf),
        "wv": wv_h.astype(bf),
        "wo": wo_h.astype(bf),
        "trim": trim.astype(bf),
    }

    in_maps = []
    for c in range(NCORES):
        xs = x[c * BL : (c + 1) * BL] * sX  # [BL, S, D]
        xt = np.zeros((BL, D, SP), np.float32)
        xt[:, :, 2 : 2 + S] = xs.transpose(0, 2, 1)
        xh = _q8(xt)
        xl = _q8(xt - xh.astype(np.float32))
        m = dict(consts)
        m["xhi"] = np.ascontiguousarray(
            xh.reshape(BL, NIC, P, SP).transpose(0, 2, 1, 3)
        )
        m["xlo"] = np.ascontiguousarray(
            xl.reshape(BL, NIC, P, SP).transpose(0, 2, 1, 3)
        )
        in_maps.append(m)
    return in_maps


_NC_CACHE = {}


def get_nc():
    if "nc" not in _NC_CACHE:
        _NC_CACHE["nc"] = build_nc()
    return _NC_CACHE["nc"]


def kernel(**inputs):
    nc = get_nc()
    in_maps = prep_inputs(inputs)
    res = run_bass_kernel_spmd(nc, in_maps, list(range(NCORES)))
    outs = [np.asarray(r["out"], np.float32) for r in res.results]
    return np.concatenate(outs, axis=0)


if __name__ == "__main__":
    nc = build_nc()
    print("built ok")


# revision 36
# speedup vs baseline: 1.0224x; 1.0224x over previous
"""Trainium2 Bass kernel for nn_BAKTTime: causal-conv frequency layer + LN + causal MHA.

Sharding: pure data-parallel over batch - 8 of the 64 batch items per NeuronCore,
no collectives. Each core runs a 3-stage software-pipelined program
[conv+LN(b) | qkv+attention+normalize(b-1) with the out-projection of b-2
interleaved at head-pair boundaries] over its 8 batch items
(S=512, D=512, H=8, DK=64).

Key structure:
  - conv runs as fp8e4 DoubleRow matmuls (contract 256/instr, 0.5 cy/row) with
    a hi+lo residual split of both x and the folded conv weights (3 of the 4
    cross terms; dropped lo*lo ~ 2^-8 relative, so conv accuracy ~ bf16).
    Splits are host-prepped; LN absorbs the fp8 scale factors. SP=640 keeps
    the DR k-tile stride a multiple of 128 (walrus s3_lw_dual_fp8 check).
    Conv stream cost drops 24576 -> 18432 PE rows per batch.
  - batched DMAs (HWDGE fixed cost is ~632ns/DMA): 1 xt-hi + 1 xt-lo load,
    4 batched h-transposes (3D-out xbar transpose -> htall[dc, i, s]),
    1 denominator broadcast per head pair, 1 odd-head shift, 1 output store
    per batch (the last batch stores per s-tile to shorten the drain).
  - denominator chain per head pair: ctx+denominator rows staged off PSUM on
    DVE (+TINY only on the denominator partition via a [65,1] bias vector),
    one [1,2,S]->[64,2,S] f32 broadcast DMA, reciprocal_approx_fast in place
    on the broadcasted tile (a partition-64-based DVE write lands on the
    wrong partitions, so never reciprocal the raw row), then the normalize
    muls into the head-pair tile cpack.
  - engine balance per batch (~ns, PE-bound): PE ~29900 (71680 rows),
    DVE ~22000 (bn stats, q/k PSUM drains, chains), ACT ~20000 (softmax exp,
    conv-PSUM staging, v drains, rstd, half the out-proj drains),
    Pool ~11000 (h-pass, probs trim muls). The out-proj s-tiles of b-2 are
    interleaved at pair boundaries so PE stays busy while the chains run.
"""

import sys

sys.path.insert(0, "/opt/trn_rl_repo")

import numpy as np
import ml_dtypes
from contextlib import ExitStack

import concourse.bass as bass
from concourse import bacc
import concourse.mybir as mybir
import concourse.tile as tile
from concourse.bass_utils import run_bass_kernel_spmd

# Force Exp and Ln to resolve to the single table set that contains both
# (natural_log_exp_and_others), so ACT doesn't thrash table loads.
import concourse.hw_specs as _hw_specs

_orig_get_tables = _hw_specs.get_activation_tables


def _patched_get_tables(arch):
    t = dict(_orig_get_tables(arch))
    exp = mybir.ActivationFunctionType.Exp
    ln = mybir.ActivationFunctionType.Ln
    for name, funcs in t.items():
        if name != "natural_log_exp_and_others" and (exp in funcs or ln in funcs):
            t[name] = funcs - {exp, ln}
    return t


_hw_specs.get_activation_tables = _patched_get_tables
bacc.get_activation_tables = _patched_get_tables

B, S, D, H, KW = 64, 512, 512, 8, 3
DK = D // H  # 64
NCORES = 8
BL = B // NCORES  # 8 batches per core
P = 128
NST = S // P  # 4 s-tiles
NIC = D // P  # 4 input-chunks
NHP = H // 2  # 4 head pairs
SP = 640  # padded xT free dim: 2 zero cols + 512 data + 126 pad (mult of 128)
EPS = 1e-12
TINY = 2e-5  # q=0 denominator guard; 1/TINY = 5e4 < f16 max, and the
# +TINY bias the staging copy adds to ctx rows is negligible (~2e-5 abs)
F32 = mybir.dt.float32
F16 = mybir.dt.float16
BF16 = mybir.dt.bfloat16
F8 = mybir.dt.float8e4
DRM = mybir.MatmulPerfMode.DoubleRow
AF = mybir.ActivationFunctionType
NCG = 6  # conv DR groups: (tap k, chunk-pair cp)


def build_nc():
    nc = bacc.Bacc("TRN2", target_bir_lowering=False)
    xhi = nc.declare_dram_parameter("xhi", [BL, P, NIC, SP], F8, isOutput=False)
    xlo = nc.declare_dram_parameter("xlo", [BL, P, NIC, SP], F8, isOutput=False)
    whi = nc.declare_dram_parameter("whi", [P, NCG, 2, D], F8, isOutput=False)
    wlo = nc.declare_dram_parameter("wlo", [P, NCG, 2, D], F8, isOutput=False)
    wq = nc.declare_dram_parameter("wq", [NIC, P, D], BF16, isOutput=False)
    wk = nc.declare_dram_parameter("wk", [NIC, P, D], BF16, isOutput=False)
    wv = nc.declare_dram_parameter("wv", [NIC, P, D], BF16, isOutput=False)
    wo = nc.declare_dram_parameter("wo", [NIC, P, D], BF16, isOutput=False)
    trim = nc.declare_dram_parameter("trim", [P, 2, P], BF16, isOutput=False)
    out = nc.declare_dram_parameter("out", [BL, S, D], F32, isOutput=True)

    with ExitStack() as ctx:
        tc = ctx.enter_context(tile.TileContext(nc))
        singles = ctx.enter_context(tc.tile_pool(name="singles", bufs=1))
        xt_pool = ctx.enter_context(tc.tile_pool(name="xt", bufs=2))
        a_pool = ctx.enter_context(tc.tile_pool(name="a", bufs=6))
        stat_pool = ctx.enter_context(tc.tile_pool(name="stat", bufs=4))
        h_pool = ctx.enter_context(tc.tile_pool(name="h", bufs=8))
        ht_pool = ctx.enter_context(tc.tile_pool(name="ht", bufs=2))
        qk_pool = ctx.enter_context(tc.tile_pool(name="qk", bufs=16))
        v_pool = ctx.enter_context(tc.tile_pool(name="v", bufs=8))
        pt_pool = ctx.enter_context(tc.tile_pool(name="pt", bufs=6))
        dn_pool = ctx.enter_context(tc.tile_pool(name="dn", bufs=3))
        r_pool = ctx.enter_context(tc.tile_pool(name="r", bufs=3))
        cx_pool = ctx.enter_context(tc.tile_pool(name="cx", bufs=4))
        cp_pool = ctx.enter_context(tc.tile_pool(name="cp", bufs=3))
        o_pool = ctx.enter_context(tc.tile_pool(name="o", bufs=2))
        ps_a = ctx.enter_context(tc.tile_pool(name="ps_a", bufs=2, space="PSUM"))
        ps_mm = ctx.enter_context(tc.tile_pool(name="ps_mm", bufs=2, space="PSUM"))
        ps_sc = ctx.enter_context(tc.tile_pool(name="ps_sc", bufs=1, space="PSUM"))
        ps_cx = ctx.enter_context(tc.tile_pool(name="ps_cx", bufs=2, space="PSUM"))

        # --- load weights once; first-batch critical path (xh, xl, whi_a,
        # wlo_a) leads on the two HWDGE queues, the rest follows ---
        whi_sb = [singles.tile([P, 3, 2, D], F8, name=f"whi{i}", tag=f"whi{i}")
                  for i in range(2)]
        wlo_sb = [singles.tile([P, 3, 2, D], F8, name=f"wlo{i}", tag=f"wlo{i}")
                  for i in range(2)]
        wq_sb = [singles.tile([P, D], BF16, name=f"wq{i}", tag=f"wq{i}") for i in range(NIC)]
        wk_sb = [singles.tile([P, D], BF16, name=f"wk{i}", tag=f"wk{i}") for i in range(NIC)]
        wv_sb = [singles.tile([P, D], BF16, name=f"wv{i}", tag=f"wv{i}") for i in range(NIC)]
        wo_sb = [singles.tile([P, D], BF16, name=f"wo{i}", tag=f"wo{i}") for i in range(NIC)]
        trim_sb = singles.tile([P, 2, P], BF16, name="trim", tag="trim")
        eps_sb = singles.tile([P, 1], F32, name="eps", tag="eps")
        nc.vector.memset(eps_sb, EPS)
        tiny_sb = singles.tile([P, 1], F32, name="tiny", tag="tiny")
        nc.vector.memset(tiny_sb, TINY)
        zero_sb = singles.tile([P, 1], F32, name="zero", tag="zero")
        nc.vector.memset(zero_sb, 0.0)
        # bias vector for the ctx+denominator staging copy: 0 on the 64 ctx
        # partitions, TINY on the denominator row (partition 64)
        tiny64 = singles.tile([65, 1], F32, name="tiny64", tag="tiny64")
        nc.vector.memset(tiny64[0:64, :], 0.0)
        nc.vector.memset(tiny64[64:65, :], TINY)

        def load_xt(b, lead=False):
            xh = xt_pool.tile([P, NIC, SP], F8, name="xh", tag="xh")
            xl = xt_pool.tile([P, NIC, SP], F8, name="xl", tag="xl")
            nc.sync.dma_start(out=xh, in_=xhi[b])
            (nc.scalar if lead else nc.sync).dma_start(out=xl, in_=xlo[b])
            return (xh, xl)

        xt_cur = load_xt(0, lead=True)
        nc.sync.dma_start(out=whi_sb[0], in_=whi[:, 0:3])
        nc.scalar.dma_start(out=wlo_sb[0], in_=wlo[:, 0:3])
        nc.sync.dma_start(out=whi_sb[1], in_=whi[:, 3:6])
        nc.scalar.dma_start(out=wlo_sb[1], in_=wlo[:, 3:6])

        def load_proj_weights():
            # issued after front(0): needed only by mid(0)
            nc.gpsimd.dma_start(out=trim_sb, in_=trim[:])
            for i in range(NIC):
                nc.gpsimd.dma_start(out=wq_sb[i], in_=wq[i])
                nc.gpsimd.dma_start(out=wk_sb[i], in_=wk[i])
            for i in range(NIC):
                nc.scalar.dma_start(out=wv_sb[i], in_=wv[i])
                nc.gpsimd.dma_start(out=wo_sb[i], in_=wo[i])

        def conv_lhs(xt_sb, st, g):
            # x-window AP for DR group g=(k, cp): partitions = chan-in-chunk,
            # j in {0,1} selects chunk 2cp+j, free = 128 s-window cols.
            k, cp = g // 2, g % 2
            base = (2 * cp) * SP + st * P + k
            t = xt_sb[:, 0, 0]
            return bass.AP(
                tensor=t.tensor,
                offset=t.offset + base,
                ap=[[t.ap[0][0], P], [SP, 2], [1, P]],
            )

        def front(b, xt_pair):
            """conv (fp8 DR, 3 hi/lo terms) + LN + batched h-transpose."""
            xh, xl = xt_pair
            mv = stat_pool.tile([P, NST, 2], F32, name="mv", tag="mv")
            lnv = stat_pool.tile([P, NST], F32, name="lnv", tag="lnv")
            rstd = stat_pool.tile([P, NST], F32, name="rstd", tag="rstd")
            htall = ht_pool.tile([P, NIC, S], BF16, name="htall", tag="htall")
            a_list = []
            terms = ((xh, whi_sb), (xh, wlo_sb), (xl, whi_sb))
            for st in range(NST):
                aps = ps_a.tile([P, D], F32, name="aps", tag="aps")
                n = 0
                for xt_sb, w_sb in terms:
                    for g in range(NCG):
                        nc.tensor.matmul(
                            aps,
                            lhsT=conv_lhs(xt_sb, st, g),
                            rhs=w_sb[g // 3][:, g % 3, :, :],
                            start=(n == 0),
                            stop=(n == 3 * NCG - 1),
                            perf_mode=DRM,
                        )
                        n += 1
                asb = a_pool.tile([P, D], F32, name="asb", tag="asb")
                nc.scalar.copy(asb, aps)
                stats = stat_pool.tile([P, 6], F32, name="bnst", tag="bnst")
                nc.vector.bn_stats(out=stats, in_=asb)
                nc.vector.bn_aggr(out=mv[:, st, :], in_=stats)
                a_list.append(asb)
            nc.scalar.activation(lnv, mv[:, :, 1], AF.Ln, bias=eps_sb, scale=1.0)
            nc.scalar.activation(rstd, lnv, AF.Exp, bias=zero_sb, scale=-0.5)
            for st in range(NST):
                hsb = h_pool.tile([P, D], BF16, name="hsb", tag="hsb")
                nc.vector.tensor_scalar(
                    hsb,
                    a_list[st],
                    scalar1=mv[:, st, 0:1],
                    scalar2=rstd[:, st : st + 1],
                    op0=mybir.AluOpType.subtract,
                    op1=mybir.AluOpType.mult,
                )
                nc.sync.dma_start(
                    out=htall[:, :, st * P : (st + 1) * P], in_=hsb, transpose=True
                )
            return htall

        def mid(b, htall, prev_tail, last=False):
            """projections + attention + per-pair normalize for batch b;
            out-proj s-tiles of prev_tail=(b-2, cpack, oall) are interleaved
            at head-pair boundaries so PE stays busy while ACT drains the
            pair-boundary copies."""
            qt_sb = []
            kt_sb = []
            for oc in range(NIC):
                qps = ps_mm.tile([P, S], F32, name="qps", tag="qps")
                for i in range(NIC):
                    nc.tensor.matmul(
                        qps,
                        lhsT=wq_sb[i][:, oc * P : (oc + 1) * P],
                        rhs=htall[:, i, :],
                        start=(i == 0),
                        stop=(i == NIC - 1),
                    )
                qsb = qk_pool.tile([P, S], BF16, name="qtsb", tag="qtsb")
                nc.vector.tensor_copy(qsb, qps)
                qt_sb.append(qsb)

                kps = ps_mm.tile([P, S], F32, name="qps", tag="qps")
                for i in range(NIC):
                    nc.tensor.matmul(
                        kps,
                        lhsT=wk_sb[i][:, oc * P : (oc + 1) * P],
                        rhs=htall[:, i, :],
                        start=(i == 0),
                        stop=(i == NIC - 1),
                    )
                ksb = qk_pool.tile([P, S], BF16, name="qtsb", tag="qtsb")
                nc.vector.tensor_copy(ksb, kps)
                kt_sb.append(ksb)

            v_aug = []
            for st in range(NST):
                vps = ps_mm.tile([P, D], F32, name="qps", tag="qps")
                for i in range(NIC):
                    nc.tensor.matmul(
                        vps,
                        lhsT=htall[:, i, st * P : (st + 1) * P],
                        rhs=wv_sb[i],
                        start=(i == 0),
                        stop=(i == NIC - 1),
                    )
                vsb = v_pool.tile([P, H, 66], BF16, name="vsb", tag="vsb")
                nc.vector.memset(vsb[:, :, 64:66], 1.0)
                nc.vector.tensor_copy(
                    vsb[:, :, 0:64], vps.rearrange("p (h d) -> p h d", h=H)
                )
                v_aug.append(vsb)

            cpack = cp_pool.tile([P, NHP, S], BF16, name="cpack", tag="cpack")
            ctmp = cp_pool.tile([DK, NHP, S], BF16, name="ctmp", tag="ctmp")

            def tail_mm_st(st):
                if prev_tail is None:
                    return
                tb, tcpack, toall = prev_tail
                ops = ps_mm.tile([P, D], F32, name="qps", tag="qps")
                for hp in range(NHP):
                    nc.tensor.matmul(
                        ops,
                        lhsT=tcpack[:, hp, st * P : (st + 1) * P],
                        rhs=wo_sb[hp],
                        start=(hp == 0),
                        stop=(hp == NHP - 1),
                    )
                if st % 2 == 0:
                    nc.scalar.copy(toall[:, st, :], ops)
                else:
                    nc.vector.tensor_copy(toall[:, st, :], ops)
                if st == NST - 1:
                    nc.sync.dma_start(
                        out=out[tb].rearrange("(st p) d -> p st d", st=NST),
                        in_=toall,
                    )

            def make_trim(pt, ki):
                tsl = trim_sb[:, 1 if ki == 0 else 0, :]
                tbc = bass.AP(
                    tensor=tsl.tensor,
                    offset=tsl.offset,
                    ap=[tsl.ap[0], [0, 2], [1, P]],
                )
                nc.vector.tensor_mul(pt[:, :, 0:P], pt[:, :, 0:P], tbc)

            for hp in range(NHP):
                cps2 = [
                    ps_cx.tile([65, S], F32, name="cps", tag="cps") for _ in range(2)
                ]
                for ki in range(NST):
                    qoff = ki * P
                    nq = S - qoff
                    sps = ps_sc.tile([P, 2, S], F32, name="sps", tag="sps")
                    for e in range(2):
                        hr = e * DK
                        nc.tensor.matmul(
                            sps[:, e, 0:nq],
                            lhsT=kt_sb[hp][hr : hr + DK, ki * P : (ki + 1) * P],
                            rhs=qt_sb[hp][hr : hr + DK, qoff:S],
                            start=True,
                            stop=True,
                        )
                    pt = pt_pool.tile([P, 2, S], BF16, name="pt", tag="pt")
                    nc.scalar.activation(
                        pt[:, :, 0:nq], sps[:, :, 0:nq], AF.Exp, scale=0.125
                    )
                    make_trim(pt, ki)
                    for e in range(2):
                        nc.tensor.matmul(
                            cps2[e][:, qoff:S],
                            lhsT=v_aug[ki][:, 2 * hp + e, 0:65],
                            rhs=pt[:, e, 0:nq],
                            start=(ki == 0),
                            stop=(ki == NST - 1),
                        )
                # pair boundary: stage ctx+denominator rows off PSUM on DVE
                # (+TINY only on the denominator partition), interleave the
                # out-proj s-tile of b-2, then denominator chain.
                cxp = cx_pool.tile([65, 2, S], F32, name="cxp", tag="cxp")
                for e in range(2):
                    nc.vector.tensor_scalar_add(cxp[:, e, :], cps2[e], tiny64)
                tail_mm_st(hp)
                rall = r_pool.tile([DK, 2, S], F32, name="rall", tag="rall")
                rsrc = cxp[64:65, 0, 0]
                rsrc = bass.AP(
                    tensor=rsrc.tensor,
                    offset=rsrc.offset,
                    ap=[[rsrc.ap[0][0], 1], [0, DK], [S, 2], [1, S]],
                )
                nc.sync.dma_start(out=rall, in_=rsrc)
                nc.vector.reciprocal_approx_fast(out=rall, in_=rall)
                nc.vector.tensor_mul(cpack[0:DK, hp, :], cxp[0:DK, 0, :], rall[:, 0, :])
                nc.vector.tensor_mul(ctmp[:, hp, :], cxp[0:DK, 1, :], rall[:, 1, :])
            nc.sync.dma_start(out=cpack[DK:P, :, :], in_=ctmp)
            oall = o_pool.tile([P, NST, D], F32, name="oall", tag="oall")
            return (b, cpack, oall)

        def tail_mm_solo(prev_tail):
            tb, tcpack, toall = prev_tail
            for st in range(NST):
                ops = ps_mm.tile([P, D], F32, name="qps", tag="qps")
                for hp in range(NHP):
                    nc.tensor.matmul(
                        ops,
                        lhsT=tcpack[:, hp, st * P : (st + 1) * P],
                        rhs=wo_sb[hp],
                        start=(hp == 0),
                        stop=(hp == NHP - 1),
                    )
                if st % 2 == 0:
                    nc.scalar.copy(toall[:, st, :], ops)
                else:
                    nc.vector.tensor_copy(toall[:, st, :], ops)
                nc.sync.dma_start(
                    out=out[tb, st * P : (st + 1) * P, :], in_=toall[:, st, :]
                )

        # 3-stage pipeline: [front(b) | mid(b-1) w/ interleaved out-proj(b-2)]
        pend_mid = None
        pend_tail = None
        for b in range(BL):
            xt_next = load_xt(b + 1) if b + 1 < BL else None
            ht = front(b, xt_cur)
            if b == 0:
                load_proj_weights()
            if pend_mid is not None:
                pend_tail = mid(*pend_mid, pend_tail)
            pend_mid = (b, ht)
            xt_cur = xt_next
        pend_tail = mid(*pend_mid, pend_tail, last=True)
        tail_mm_solo(pend_tail)

    nc.compile()
    return nc


def _q8(a):
    """fp8e4 round-to-nearest via ml_dtypes."""
    return a.astype(ml_dtypes.float8_e4m3)


def prep_inputs(inputs):
    """Host-side prep: shard over batch, fold scales into weights, fp8 hi/lo
    splits of x and conv weights, pre-transpose x."""
    x = np.asarray(inputs["x"], np.float32)
    conv_w = np.asarray(inputs["conv_w"], np.float32)
    conv_b = np.asarray(inputs["conv_b"], np.float32)
    sb = np.asarray(inputs["sqrt_beta"], np.float32).reshape(D)
    ln_w = np.asarray(inputs["ln_w"], np.float32)
    ln_b = np.asarray(inputs["ln_b"], np.float32)
    Wq = np.asarray(inputs["Wq"], np.float32)
    Wk = np.asarray(inputs["Wk"], np.float32)
    Wv = np.asarray(inputs["Wv"], np.float32)
    Wo = np.asarray(inputs["Wo"], np.float32)
    mask = np.asarray(inputs["mask"])

    for nm in ("bq", "bk", "bv", "bo"):
        assert not np.any(np.asarray(inputs[nm])), f"{nm} must be zero"
    assert not np.any(conv_b), "conv_b must be zero"
    assert not np.any(ln_b), "ln_b must be zero"
    assert np.array_equal(
        mask.reshape(S, S), np.tril(np.ones((S, S), mask.dtype))
    ), "mask must be causal"

    c1 = 1.0 - sb * sb
    c2 = 1.0 + sb * sb
    Wp = conv_w * c1[:, None, None]  # [o, i, k]
    Wp[np.arange(D), np.arange(D), 2] += c2

    # fp8 scales (power of 2, amax -> ~240)
    sW = 2.0 ** np.floor(np.log2(240.0 / np.abs(Wp).max()))
    sX = 2.0 ** np.floor(np.log2(240.0 / np.abs(x).max()))

    # weights: w[p, (k, cp), j, o] = Wp[o, (2cp+j)*128+p, k] * sW, hi + lo
    Ws = Wp * sW  # [o, i, k]
    wt = Ws.transpose(1, 2, 0).reshape(NIC, P, KW, D)  # [i-chunk, p, k, o]
    warr = np.empty((P, NCG, 2, D), np.float32)
    for k in range(KW):
        for cp in range(2):
            for j in range(2):
                warr[:, k * 2 + cp, j, :] = wt[2 * cp + j, :, k, :]
    whi = _q8(warr)
    wlo = _q8(warr - whi.astype(np.float32))

    def fold(W):  # [o, i] -> [ic, il, o] with ln_w folded on i
        Wf = W * ln_w[None, :]
        return np.ascontiguousarray(Wf.T).reshape(NIC, P, D)

    wq_h, wk_h, wv_h = fold(Wq), fold(Wk), fold(Wv)
    wo_h = np.ascontiguousarray(Wo.T).reshape(NIC, P, D)

    tri = np.triu(np.ones((P, P), np.float32))
    tri0 = tri.copy()
    tri0[:, 0] = 0.0
    trim = np.stack([tri, tri0], axis=1)  # [P, 2, P]

    bf = ml_dtypes.bfloat16
    consts = {
        "whi": whi,
        "wlo": wlo,
        "wq": wq_h.astype(bf),
        "wk": wk_h.astype(bf),
        "wv": wv_h.astype(bf),
        "wo": wo_h.astype(bf),
        "trim": trim.astype(bf),
    }

    in_maps = []
    for c in range(NCORES):
        xs = x[c * BL : (c + 1) * BL] * sX  # [BL, S, D]
        xt = np.zeros((BL, D, SP), np.float32)
        xt[:, :, 2 : 2 + S] = xs.transpose(0, 2, 1)
        xh = _q8(xt)
        xl = _q8(xt - xh.astype(np.float32))
        m = dict(consts)
        m["xhi"] = np.ascontiguousarray(
            xh.reshape(BL, NIC, P, SP).transpose(0, 2, 1, 3)
        )
        m["xlo"] = np.ascontiguousarray(
            xl.reshape(BL, NIC, P, SP).transpose(0, 2, 1, 3)
        )
        in_maps.append(m)
    return in_maps


_NC_CACHE = {}


def get_nc():
    if "nc" not in _NC_CACHE:
        _NC_CACHE["nc"] = build_nc()
    return _NC_CACHE["nc"]


def kernel(**inputs):
    nc = get_nc()
    in_maps = prep_inputs(inputs)
    res = run_bass_kernel_spmd(nc, in_maps, list(range(NCORES)))
    outs = [np.asarray(r["out"], np.float32) for r in res.results]
    return np.concatenate(outs, axis=0)


if __name__ == "__main__":
    nc = build_nc()
    print("built ok")


# revision 37
# speedup vs baseline: 1.0494x; 1.0265x over previous
"""Trainium2 Bass kernel for nn_BAKTTime: causal-conv frequency layer + LN + causal MHA.

Sharding: pure data-parallel over batch - 8 of the 64 batch items per NeuronCore,
no collectives. Each core runs a 3-stage software-pipelined program
[conv+LN(b) | qkv+attention+normalize(b-1) with the out-projection of b-2
interleaved at head-pair boundaries] over its 8 batch items
(S=512, D=512, H=8, DK=64).

Key structure:
  - conv runs as fp8e4 DoubleRow matmuls (contract 256/instr, 0.5 cy/row) with
    a hi+lo residual split of both x and the folded conv weights (3 of the 4
    cross terms; dropped lo*lo ~ 2^-8 relative, so conv accuracy ~ bf16).
    Splits are host-prepped; LN absorbs the fp8 scale factors. SP=640 keeps
    the DR k-tile stride a multiple of 128 (walrus s3_lw_dual_fp8 check).
    Conv stream cost drops 24576 -> 18432 PE rows per batch.
  - batched DMAs (HWDGE fixed cost is ~632ns/DMA): 1 xt-hi + 1 xt-lo load,
    4 batched h-transposes (3D-out xbar transpose -> htall[dc, i, s]),
    1 denominator broadcast per head pair, 1 odd-head shift, 1 output store
    per batch (the last batch stores per s-tile to shorten the drain).
  - denominator chain per head pair: ctx+denominator rows staged off PSUM on
    DVE (+TINY only on the denominator partition via a [65,1] bias vector),
    one [1,2,S]->[64,2,S] f32 broadcast DMA, reciprocal_approx_fast in place
    on the broadcasted tile (a partition-64-based DVE write lands on the
    wrong partitions, so never reciprocal the raw row), then the normalize
    muls into the head-pair tile cpack.
  - engine balance per batch (~ns, PE-bound): PE ~29900 (71680 rows),
    DVE ~22000 (bn stats, q/k PSUM drains, chains), ACT ~20000 (softmax exp,
    conv-PSUM staging, v drains, rstd, half the out-proj drains),
    Pool ~11000 (h-pass, probs trim muls). The out-proj s-tiles of b-2 are
    interleaved at pair boundaries so PE stays busy while the chains run.
"""

import sys

sys.path.insert(0, "/opt/trn_rl_repo")

import numpy as np
import ml_dtypes
from contextlib import ExitStack

import concourse.bass as bass
from concourse import bacc
import concourse.mybir as mybir
import concourse.tile as tile
from concourse.bass_utils import run_bass_kernel_spmd

# Force Exp and Ln to resolve to the single table set that contains both
# (natural_log_exp_and_others), so ACT doesn't thrash table loads.
import concourse.hw_specs as _hw_specs

_orig_get_tables = _hw_specs.get_activation_tables


def _patched_get_tables(arch):
    t = dict(_orig_get_tables(arch))
    exp = mybir.ActivationFunctionType.Exp
    ln = mybir.ActivationFunctionType.Ln
    for name, funcs in t.items():
        if name != "natural_log_exp_and_others" and (exp in funcs or ln in funcs):
            t[name] = funcs - {exp, ln}
    return t


_hw_specs.get_activation_tables = _patched_get_tables
bacc.get_activation_tables = _patched_get_tables

B, S, D, H, KW = 64, 512, 512, 8, 3
DK = D // H  # 64
NCORES = 8
BL = B // NCORES  # 8 batches per core
P = 128
NST = S // P  # 4 s-tiles
NIC = D // P  # 4 input-chunks
NHP = H // 2  # 4 head pairs
SP = 640  # padded xT free dim: 2 zero cols + 512 data + 126 pad (mult of 128)
EPS = 1e-12
TINY = 2e-5  # q=0 denominator guard; 1/TINY = 5e4 < f16 max, and the
# +TINY bias the staging copy adds to ctx rows is negligible (~2e-5 abs)
F32 = mybir.dt.float32
F16 = mybir.dt.float16
BF16 = mybir.dt.bfloat16
F8 = mybir.dt.float8e4
DRM = mybir.MatmulPerfMode.DoubleRow
AF = mybir.ActivationFunctionType
NCG = 6  # conv DR groups: (tap k, chunk-pair cp)


def build_nc():
    nc = bacc.Bacc("TRN2", target_bir_lowering=False)
    xhi = nc.declare_dram_parameter("xhi", [BL, P, NIC, SP], F8, isOutput=False)
    xlo = nc.declare_dram_parameter("xlo", [BL, P, NIC, SP], F8, isOutput=False)
    whi = nc.declare_dram_parameter("whi", [P, NCG, 2, D], F8, isOutput=False)
    wlo = nc.declare_dram_parameter("wlo", [P, NCG, 2, D], F8, isOutput=False)
    wq = nc.declare_dram_parameter("wq", [NIC, P, D], BF16, isOutput=False)
    wk = nc.declare_dram_parameter("wk", [NIC, P, D], BF16, isOutput=False)
    wv = nc.declare_dram_parameter("wv", [NIC, P, D], BF16, isOutput=False)
    wo = nc.declare_dram_parameter("wo", [NIC, P, D], BF16, isOutput=False)
    trim = nc.declare_dram_parameter("trim", [P, 2, P], BF16, isOutput=False)
    out = nc.declare_dram_parameter("out", [BL, S, D], F32, isOutput=True)

    with ExitStack() as ctx:
        tc = ctx.enter_context(tile.TileContext(nc))
        singles = ctx.enter_context(tc.tile_pool(name="singles", bufs=1))
        xt_pool = ctx.enter_context(tc.tile_pool(name="xt", bufs=2))
        a_pool = ctx.enter_context(tc.tile_pool(name="a", bufs=6))
        stat_pool = ctx.enter_context(tc.tile_pool(name="stat", bufs=4))
        h_pool = ctx.enter_context(tc.tile_pool(name="h", bufs=8))
        ht_pool = ctx.enter_context(tc.tile_pool(name="ht", bufs=2))
        qk_pool = ctx.enter_context(tc.tile_pool(name="qk", bufs=16))
        v_pool = ctx.enter_context(tc.tile_pool(name="v", bufs=8))
        pt_pool = ctx.enter_context(tc.tile_pool(name="pt", bufs=6))
        dn_pool = ctx.enter_context(tc.tile_pool(name="dn", bufs=3))
        r_pool = ctx.enter_context(tc.tile_pool(name="r", bufs=3))
        cx_pool = ctx.enter_context(tc.tile_pool(name="cx", bufs=4))
        cp_pool = ctx.enter_context(tc.tile_pool(name="cp", bufs=3))
        o_pool = ctx.enter_context(tc.tile_pool(name="o", bufs=2))
        ps_a = ctx.enter_context(tc.tile_pool(name="ps_a", bufs=2, space="PSUM"))
        ps_mm = ctx.enter_context(tc.tile_pool(name="ps_mm", bufs=2, space="PSUM"))
        ps_sc = ctx.enter_context(tc.tile_pool(name="ps_sc", bufs=1, space="PSUM"))
        ps_cx = ctx.enter_context(tc.tile_pool(name="ps_cx", bufs=2, space="PSUM"))

        # --- load weights once; first-batch critical path (xh, xl, whi_a,
        # wlo_a) leads on the two HWDGE queues, the rest follows ---
        whi_sb = [singles.tile([P, 3, 2, D], F8, name=f"whi{i}", tag=f"whi{i}")
                  for i in range(2)]
        wlo_sb = [singles.tile([P, 3, 2, D], F8, name=f"wlo{i}", tag=f"wlo{i}")
                  for i in range(2)]
        wq_sb = [singles.tile([P, D], BF16, name=f"wq{i}", tag=f"wq{i}") for i in range(NIC)]
        wk_sb = [singles.tile([P, D], BF16, name=f"wk{i}", tag=f"wk{i}") for i in range(NIC)]
        wv_sb = [singles.tile([P, D], BF16, name=f"wv{i}", tag=f"wv{i}") for i in range(NIC)]
        wo_sb = [singles.tile([P, D], BF16, name=f"wo{i}", tag=f"wo{i}") for i in range(NIC)]
        trim_sb = singles.tile([P, 2, P], BF16, name="trim", tag="trim")
        eps_sb = singles.tile([P, 1], F32, name="eps", tag="eps")
        nc.vector.memset(eps_sb, EPS)
        tiny_sb = singles.tile([P, 1], F32, name="tiny", tag="tiny")
        nc.vector.memset(tiny_sb, TINY)
        zero_sb = singles.tile([P, 1], F32, name="zero", tag="zero")
        nc.vector.memset(zero_sb, 0.0)
        # bias vector for the ctx+denominator staging copy: 0 on the 64 ctx
        # partitions, TINY on the denominator row (partition 64)
        tiny64 = singles.tile([65, 1], F32, name="tiny64", tag="tiny64")
        nc.vector.memset(tiny64[0:64, :], 0.0)
        nc.vector.memset(tiny64[64:65, :], TINY)

        def load_xt(b, lead=False):
            xh = xt_pool.tile([P, NIC, SP], F8, name="xh", tag="xh")
            xl = xt_pool.tile([P, NIC, SP], F8, name="xl", tag="xl")
            nc.sync.dma_start(out=xh, in_=xhi[b])
            (nc.scalar if lead else nc.sync).dma_start(out=xl, in_=xlo[b])
            return (xh, xl)

        xt_cur = load_xt(0, lead=True)
        nc.sync.dma_start(out=whi_sb[0], in_=whi[:, 0:3])
        nc.scalar.dma_start(out=wlo_sb[0], in_=wlo[:, 0:3])
        nc.sync.dma_start(out=whi_sb[1], in_=whi[:, 3:6])
        nc.scalar.dma_start(out=wlo_sb[1], in_=wlo[:, 3:6])

        def load_proj_weights():
            # issued after front(0): needed only by mid(0)
            nc.gpsimd.dma_start(out=trim_sb, in_=trim[:])
            for i in range(NIC):
                nc.gpsimd.dma_start(out=wq_sb[i], in_=wq[i])
                nc.gpsimd.dma_start(out=wk_sb[i], in_=wk[i])
            for i in range(NIC):
                nc.scalar.dma_start(out=wv_sb[i], in_=wv[i])
                nc.gpsimd.dma_start(out=wo_sb[i], in_=wo[i])

        def conv_lhs(xt_sb, st, g):
            # x-window AP for DR group g=(k, cp): partitions = chan-in-chunk,
            # j in {0,1} selects chunk 2cp+j, free = 128 s-window cols.
            k, cp = g // 2, g % 2
            base = (2 * cp) * SP + st * P + k
            t = xt_sb[:, 0, 0]
            return bass.AP(
                tensor=t.tensor,
                offset=t.offset + base,
                ap=[[t.ap[0][0], P], [SP, 2], [1, P]],
            )

        def front(b, xt_pair):
            """conv (fp8 DR, 3 hi/lo terms) + LN + batched h-transpose."""
            xh, xl = xt_pair
            mv = stat_pool.tile([P, NST, 2], F32, name="mv", tag="mv")
            lnv = stat_pool.tile([P, NST], F32, name="lnv", tag="lnv")
            rstd = stat_pool.tile([P, NST], F32, name="rstd", tag="rstd")
            htall = ht_pool.tile([P, NIC, S], BF16, name="htall", tag="htall")
            a_list = []
            terms = ((xh, whi_sb), (xh, wlo_sb), (xl, whi_sb))
            for st in range(NST):
                aps = ps_a.tile([P, D], F32, name="aps", tag="aps")
                n = 0
                for xt_sb, w_sb in terms:
                    for g in range(NCG):
                        nc.tensor.matmul(
                            aps,
                            lhsT=conv_lhs(xt_sb, st, g),
                            rhs=w_sb[g // 3][:, g % 3, :, :],
                            start=(n == 0),
                            stop=(n == 3 * NCG - 1),
                            perf_mode=DRM,
                        )
                        n += 1
                asb = a_pool.tile([P, D], F32, name="asb", tag="asb")
                nc.scalar.copy(asb, aps)
                stats = stat_pool.tile([P, 6], F32, name="bnst", tag="bnst")
                nc.vector.bn_stats(out=stats, in_=asb)
                nc.vector.bn_aggr(out=mv[:, st, :], in_=stats)
                a_list.append(asb)
            nc.scalar.activation(lnv, mv[:, :, 1], AF.Ln, bias=eps_sb, scale=1.0)
            nc.scalar.activation(rstd, lnv, AF.Exp, bias=zero_sb, scale=-0.5)
            for st in range(NST):
                hsb = h_pool.tile([P, D], BF16, name="hsb", tag="hsb")
                nc.vector.tensor_scalar(
                    hsb,
                    a_list[st],
                    scalar1=mv[:, st, 0:1],
                    scalar2=rstd[:, st : st + 1],
                    op0=mybir.AluOpType.subtract,
                    op1=mybir.AluOpType.mult,
                )
                nc.sync.dma_start(
                    out=htall[:, :, st * P : (st + 1) * P], in_=hsb, transpose=True
                )
            return htall

        def mid(b, htall, prev_tail, last=False):
            """projections + attention + per-pair normalize for batch b;
            out-proj s-tiles of prev_tail=(b-2, cpack, oall) are interleaved
            at head-pair boundaries so PE stays busy while ACT drains the
            pair-boundary copies."""
            qt_sb = []
            kt_sb = []
            for oc in range(NIC):
                qps = ps_mm.tile([P, S], F32, name="qps", tag="qps")
                for i in range(NIC):
                    nc.tensor.matmul(
                        qps,
                        lhsT=wq_sb[i][:, oc * P : (oc + 1) * P],
                        rhs=htall[:, i, :],
                        start=(i == 0),
                        stop=(i == NIC - 1),
                    )
                qsb = qk_pool.tile([P, S], BF16, name="qtsb", tag="qtsb")
                nc.vector.tensor_copy(qsb, qps)
                qt_sb.append(qsb)

                kps = ps_mm.tile([P, S], F32, name="qps", tag="qps")
                for i in range(NIC):
                    nc.tensor.matmul(
                        kps,
                        lhsT=wk_sb[i][:, oc * P : (oc + 1) * P],
                        rhs=htall[:, i, :],
                        start=(i == 0),
                        stop=(i == NIC - 1),
                    )
                ksb = qk_pool.tile([P, S], BF16, name="qtsb", tag="qtsb")
                nc.vector.tensor_copy(ksb, kps)
                kt_sb.append(ksb)

            v_aug = []
            for st in range(NST):
                vps = ps_mm.tile([P, D], F32, name="qps", tag="qps")
                for i in range(NIC):
                    nc.tensor.matmul(
                        vps,
                        lhsT=htall[:, i, st * P : (st + 1) * P],
                        rhs=wv_sb[i],
                        start=(i == 0),
                        stop=(i == NIC - 1),
                    )
                vsb = v_pool.tile([P, H, 66], BF16, name="vsb", tag="vsb")
                nc.vector.memset(vsb[:, :, 64:66], 1.0)
                nc.scalar.copy(
                    vsb[:, :, 0:64], vps.rearrange("p (h d) -> p h d", h=H)
                )
                v_aug.append(vsb)

            cpack = cp_pool.tile([P, NHP, S], BF16, name="cpack", tag="cpack")
            ctmp = cp_pool.tile([DK, NHP, S], BF16, name="ctmp", tag="ctmp")

            def tail_mm_st(st):
                if prev_tail is None:
                    return
                tb, tcpack, toall = prev_tail
                ops = ps_mm.tile([P, D], F32, name="qps", tag="qps")
                for hp in range(NHP):
                    nc.tensor.matmul(
                        ops,
                        lhsT=tcpack[:, hp, st * P : (st + 1) * P],
                        rhs=wo_sb[hp],
                        start=(hp == 0),
                        stop=(hp == NHP - 1),
                    )
                if st % 2 == 0:
                    nc.scalar.copy(toall[:, st, :], ops)
                else:
                    nc.vector.tensor_copy(toall[:, st, :], ops)
                if st == NST - 1:
                    nc.sync.dma_start(
                        out=out[tb].rearrange("(st p) d -> p st d", st=NST),
                        in_=toall,
                    )

            def make_trim(pt, ki):
                tsl = trim_sb[:, 1 if ki == 0 else 0, :]
                tbc = bass.AP(
                    tensor=tsl.tensor,
                    offset=tsl.offset,
                    ap=[tsl.ap[0], [0, 2], [1, P]],
                )
                nc.vector.tensor_mul(pt[:, :, 0:P], pt[:, :, 0:P], tbc)

            for hp in range(NHP):
                cps2 = [
                    ps_cx.tile([65, S], F32, name="cps", tag="cps") for _ in range(2)
                ]
                for ki in range(NST):
                    qoff = ki * P
                    nq = S - qoff
                    sps = ps_sc.tile([P, 2, S], F32, name="sps", tag="sps")
                    for e in range(2):
                        hr = e * DK
                        nc.tensor.matmul(
                            sps[:, e, 0:nq],
                            lhsT=kt_sb[hp][hr : hr + DK, ki * P : (ki + 1) * P],
                            rhs=qt_sb[hp][hr : hr + DK, qoff:S],
                            start=True,
                            stop=True,
                        )
                    pt = pt_pool.tile([P, 2, S], BF16, name="pt", tag="pt")
                    nc.scalar.activation(
                        pt[:, :, 0:nq], sps[:, :, 0:nq], AF.Exp, scale=0.125
                    )
                    make_trim(pt, ki)
                    for e in range(2):
                        nc.tensor.matmul(
                            cps2[e][:, qoff:S],
                            lhsT=v_aug[ki][:, 2 * hp + e, 0:65],
                            rhs=pt[:, e, 0:nq],
                            start=(ki == 0),
                            stop=(ki == NST - 1),
                        )
                # pair boundary: stage ctx+denominator rows off PSUM on DVE
                # (+TINY only on the denominator partition), interleave the
                # out-proj s-tile of b-2, then denominator chain.
                cxp = cx_pool.tile([65, 2, S], F32, name="cxp", tag="cxp")
                for e in range(2):
                    nc.vector.tensor_scalar_add(cxp[:, e, :], cps2[e], tiny64)
                tail_mm_st(hp)
                rall = r_pool.tile([DK, 2, S], F32, name="rall", tag="rall")
                rsrc = cxp[64:65, 0, 0]
                rsrc = bass.AP(
                    tensor=rsrc.tensor,
                    offset=rsrc.offset,
                    ap=[[rsrc.ap[0][0], 1], [0, DK], [S, 2], [1, S]],
                )
                nc.sync.dma_start(out=rall, in_=rsrc)
                nc.vector.reciprocal_approx_fast(out=rall, in_=rall)
                nc.vector.tensor_mul(cpack[0:DK, hp, :], cxp[0:DK, 0, :], rall[:, 0, :])
                nc.vector.tensor_mul(ctmp[:, hp, :], cxp[0:DK, 1, :], rall[:, 1, :])
            nc.sync.dma_start(out=cpack[DK:P, :, :], in_=ctmp)
            oall = o_pool.tile([P, NST, D], F32, name="oall", tag="oall")
            return (b, cpack, oall)

        def tail_mm_solo(prev_tail):
            tb, tcpack, toall = prev_tail
            for st in range(NST):
                ops = ps_mm.tile([P, D], F32, name="qps", tag="qps")
                for hp in range(NHP):
                    nc.tensor.matmul(
                        ops,
                        lhsT=tcpack[:, hp, st * P : (st + 1) * P],
                        rhs=wo_sb[hp],
                        start=(hp == 0),
                        stop=(hp == NHP - 1),
                    )
                if st % 2 == 0:
                    nc.scalar.copy(toall[:, st, :], ops)
                else:
                    nc.vector.tensor_copy(toall[:, st, :], ops)
                nc.sync.dma_start(
                    out=out[tb, st * P : (st + 1) * P, :], in_=toall[:, st, :]
                )

        # 3-stage pipeline: [front(b) | mid(b-1) w/ interleaved out-proj(b-2)]
        pend_mid = None
        pend_tail = None
        for b in range(BL):
            xt_next = load_xt(b + 1) if b + 1 < BL else None
            ht = front(b, xt_cur)
            if b == 0:
                load_proj_weights()
            if pend_mid is not None:
                pend_tail = mid(*pend_mid, pend_tail)
            pend_mid = (b, ht)
            xt_cur = xt_next
        pend_tail = mid(*pend_mid, pend_tail, last=True)
        tail_mm_solo(pend_tail)

    nc.compile()
    return nc


def _q8(a):
    """fp8e4 round-to-nearest via ml_dtypes."""
    return a.astype(ml_dtypes.float8_e4m3)


def prep_inputs(inputs):
    """Host-side prep: shard over batch, fold scales into weights, fp8 hi/lo
    splits of x and conv weights, pre-transpose x."""
    x = np.asarray(inputs["x"], np.float32)
    conv_w = np.asarray(inputs["conv_w"], np.float32)
    conv_b = np.asarray(inputs["conv_b"], np.float32)
    sb = np.asarray(inputs["sqrt_beta"], np.float32).reshape(D)
    ln_w = np.asarray(inputs["ln_w"], np.float32)
    ln_b = np.asarray(inputs["ln_b"], np.float32)
    Wq = np.asarray(inputs["Wq"], np.float32)
    Wk = np.asarray(inputs["Wk"], np.float32)
    Wv = np.asarray(inputs["Wv"], np.float32)
    Wo = np.asarray(inputs["Wo"], np.float32)
    mask = np.asarray(inputs["mask"])

    for nm in ("bq", "bk", "bv", "bo"):
        assert not np.any(np.asarray(inputs[nm])), f"{nm} must be zero"
    assert not np.any(conv_b), "conv_b must be zero"
    assert not np.any(ln_b), "ln_b must be zero"
    assert np.array_equal(
        mask.reshape(S, S), np.tril(np.ones((S, S), mask.dtype))
    ), "mask must be causal"

    c1 = 1.0 - sb * sb
    c2 = 1.0 + sb * sb
    Wp = conv_w * c1[:, None, None]  # [o, i, k]
    Wp[np.arange(D), np.arange(D), 2] += c2

    # fp8 scales (power of 2, amax -> ~240)
    sW = 2.0 ** np.floor(np.log2(240.0 / np.abs(Wp).max()))
    sX = 2.0 ** np.floor(np.log2(240.0 / np.abs(x).max()))

    # weights: w[p, (k, cp), j, o] = Wp[o, (2cp+j)*128+p, k] * sW, hi + lo
    Ws = Wp * sW  # [o, i, k]
    wt = Ws.transpose(1, 2, 0).reshape(NIC, P, KW, D)  # [i-chunk, p, k, o]
    warr = np.empty((P, NCG, 2, D), np.float32)
    for k in range(KW):
        for cp in range(2):
            for j in range(2):
                warr[:, k * 2 + cp, j, :] = wt[2 * cp + j, :, k, :]
    whi = _q8(warr)
    wlo = _q8(warr - whi.astype(np.float32))

    def fold(W):  # [o, i] -> [ic, il, o] with ln_w folded on i
        Wf = W * ln_w[None, :]
        return np.ascontiguousarray(Wf.T).reshape(NIC, P, D)

    wq_h, wk_h, wv_h = fold(Wq), fold(Wk), fold(Wv)
    wo_h = np.ascontiguousarray(Wo.T).reshape(NIC, P, D)

    tri = np.triu(np.ones((P, P), np.float32))
    tri0 = tri.copy()
    tri0[:, 0] = 0.0
    trim = np.stack([tri, tri0], axis=1)  # [P, 2, P]

    bf = ml_dtypes.bfloat16
    consts = {
        "whi": whi,
        "wlo": wlo,
        "wq": wq_h.astype(bf),
        "wk": wk_h.astype(bf),
        "wv": wv_h.astype(bf),
        "wo": wo_h.astype(bf),
        "trim": trim.astype(bf),
    }

    in_maps = []
    for c in range(NCORES):
        xs = x[c * BL : (c + 1) * BL] * sX  # [BL, S, D]
        xt = np.zeros((BL, D, SP), np.float32)
        xt[:, :, 2 : 2 + S] = xs.transpose(0, 2, 1)
        xh = _q8(xt)
        xl = _q8(xt - xh.astype(np.float32))
        m = dict(consts)
        m["xhi"] = np.ascontiguousarray(
            xh.reshape(BL, NIC, P, SP).transpose(0, 2, 1, 3)
        )
        m["xlo"] = np.ascontiguousarray(
            xl.reshape(BL, NIC, P, SP).transpose(0, 2, 1, 3)
        )
        in_maps.append(m)
    return in_maps


_NC_CACHE = {}


def get_nc():
    if "nc" not in _NC_CACHE:
        _NC_CACHE["nc"] = build_nc()
    return _NC_CACHE["nc"]


def kernel(**inputs):
    nc = get_nc()
    in_maps = prep_inputs(inputs)
    res = run_bass_kernel_spmd(nc, in_maps, list(range(NCORES)))
    outs = [np.asarray(r["out"], np.float32) for r in res.results]
    return np.concatenate(outs, axis=0)


if __name__ == "__main__":
    nc = build_nc()
    print("built ok")


# revision 38
# speedup vs baseline: 1.0587x; 1.0089x over previous
"""Trainium2 Bass kernel for nn_BAKTTime: causal-conv frequency layer + LN + causal MHA.

Sharding: pure data-parallel over batch - 8 of the 64 batch items per NeuronCore,
no collectives. Each core runs a 3-stage software-pipelined program
[conv+LN(b) | qkv+attention+normalize(b-1) with the out-projection of b-2
interleaved at head-pair boundaries] over its 8 batch items
(S=512, D=512, H=8, DK=64).

Key structure:
  - conv runs as fp8e4 DoubleRow matmuls (contract 256/instr, 0.5 cy/row) with
    a hi+lo residual split of both x and the folded conv weights (3 of the 4
    cross terms; dropped lo*lo ~ 2^-8 relative, so conv accuracy ~ bf16).
    Splits are host-prepped; LN absorbs the fp8 scale factors. SP=640 keeps
    the DR k-tile stride a multiple of 128 (walrus s3_lw_dual_fp8 check).
    Conv stream cost drops 24576 -> 18432 PE rows per batch.
  - batched DMAs (HWDGE fixed cost is ~632ns/DMA): 1 xt-hi + 1 xt-lo load,
    4 batched h-transposes (3D-out xbar transpose -> htall[dc, i, s]),
    1 denominator broadcast per head pair, 1 odd-head shift, 1 output store
    per batch (the last batch stores per s-tile to shorten the drain).
  - denominator chain per head pair: ctx+denominator rows staged off PSUM on
    DVE (+TINY only on the denominator partition via a [65,1] bias vector),
    one [1,2,S]->[64,2,S] f32 broadcast DMA, reciprocal_approx_fast in place
    on the broadcasted tile (a partition-64-based DVE write lands on the
    wrong partitions, so never reciprocal the raw row), then the normalize
    muls into the head-pair tile cpack.
  - engine balance per batch (~ns, PE-bound): PE ~29900 (71680 rows),
    DVE ~22000 (bn stats, q/k PSUM drains, chains), ACT ~20000 (softmax exp,
    conv-PSUM staging, v drains, rstd, half the out-proj drains),
    Pool ~11000 (h-pass, probs trim muls). The out-proj s-tiles of b-2 are
    interleaved at pair boundaries so PE stays busy while the chains run.
"""

import sys

sys.path.insert(0, "/opt/trn_rl_repo")

import numpy as np
import ml_dtypes
from contextlib import ExitStack

import concourse.bass as bass
from concourse import bacc
import concourse.mybir as mybir
import concourse.tile as tile
from concourse.bass_utils import run_bass_kernel_spmd

# Force Exp and Ln to resolve to the single table set that contains both
# (natural_log_exp_and_others), so ACT doesn't thrash table loads.
import concourse.hw_specs as _hw_specs

_orig_get_tables = _hw_specs.get_activation_tables


def _patched_get_tables(arch):
    t = dict(_orig_get_tables(arch))
    exp = mybir.ActivationFunctionType.Exp
    ln = mybir.ActivationFunctionType.Ln
    for name, funcs in t.items():
        if name != "natural_log_exp_and_others" and (exp in funcs or ln in funcs):
            t[name] = funcs - {exp, ln}
    return t


_hw_specs.get_activation_tables = _patched_get_tables
bacc.get_activation_tables = _patched_get_tables

B, S, D, H, KW = 64, 512, 512, 8, 3
DK = D // H  # 64
NCORES = 8
BL = B // NCORES  # 8 batches per core
P = 128
NST = S // P  # 4 s-tiles
NIC = D // P  # 4 input-chunks
NHP = H // 2  # 4 head pairs
SP = 640  # padded xT free dim: 2 zero cols + 512 data + 126 pad (mult of 128)
EPS = 1e-12
TINY = 2e-5  # q=0 denominator guard; 1/TINY = 5e4 < f16 max, and the
# +TINY bias the staging copy adds to ctx rows is negligible (~2e-5 abs)
F32 = mybir.dt.float32
F16 = mybir.dt.float16
BF16 = mybir.dt.bfloat16
F8 = mybir.dt.float8e4
DRM = mybir.MatmulPerfMode.DoubleRow
AF = mybir.ActivationFunctionType
NCG = 6  # conv DR groups: (tap k, chunk-pair cp)


def build_nc():
    nc = bacc.Bacc("TRN2", target_bir_lowering=False)
    xhi = nc.declare_dram_parameter("xhi", [BL, P, NIC, SP], F8, isOutput=False)
    xlo = nc.declare_dram_parameter("xlo", [BL, P, NIC, SP], F8, isOutput=False)
    whi = nc.declare_dram_parameter("whi", [P, NCG, 2, D], F8, isOutput=False)
    wlo = nc.declare_dram_parameter("wlo", [P, NCG, 2, D], F8, isOutput=False)
    wq = nc.declare_dram_parameter("wq", [NIC, P, D], BF16, isOutput=False)
    wk = nc.declare_dram_parameter("wk", [NIC, P, D], BF16, isOutput=False)
    wv = nc.declare_dram_parameter("wv", [NIC, P, D], BF16, isOutput=False)
    wo = nc.declare_dram_parameter("wo", [NIC, P, D], BF16, isOutput=False)
    trim = nc.declare_dram_parameter("trim", [P, 2, P], BF16, isOutput=False)
    out = nc.declare_dram_parameter("out", [BL, S, D], F32, isOutput=True)

    with ExitStack() as ctx:
        tc = ctx.enter_context(tile.TileContext(nc))
        singles = ctx.enter_context(tc.tile_pool(name="singles", bufs=1))
        xt_pool = ctx.enter_context(tc.tile_pool(name="xt", bufs=2))
        a_pool = ctx.enter_context(tc.tile_pool(name="a", bufs=6))
        stat_pool = ctx.enter_context(tc.tile_pool(name="stat", bufs=4))
        h_pool = ctx.enter_context(tc.tile_pool(name="h", bufs=8))
        ht_pool = ctx.enter_context(tc.tile_pool(name="ht", bufs=2))
        qk_pool = ctx.enter_context(tc.tile_pool(name="qk", bufs=16))
        v_pool = ctx.enter_context(tc.tile_pool(name="v", bufs=8))
        pt_pool = ctx.enter_context(tc.tile_pool(name="pt", bufs=6))
        dn_pool = ctx.enter_context(tc.tile_pool(name="dn", bufs=3))
        r_pool = ctx.enter_context(tc.tile_pool(name="r", bufs=3))
        cx_pool = ctx.enter_context(tc.tile_pool(name="cx", bufs=4))
        cp_pool = ctx.enter_context(tc.tile_pool(name="cp", bufs=3))
        o_pool = ctx.enter_context(tc.tile_pool(name="o", bufs=2))
        ps_a = ctx.enter_context(tc.tile_pool(name="ps_a", bufs=2, space="PSUM"))
        ps_mm = ctx.enter_context(tc.tile_pool(name="ps_mm", bufs=2, space="PSUM"))
        ps_sc = ctx.enter_context(tc.tile_pool(name="ps_sc", bufs=1, space="PSUM"))
        ps_cx = ctx.enter_context(tc.tile_pool(name="ps_cx", bufs=2, space="PSUM"))

        # --- load weights once; first-batch critical path (xh, xl, whi_a,
        # wlo_a) leads on the two HWDGE queues, the rest follows ---
        whi_sb = [singles.tile([P, 3, 2, D], F8, name=f"whi{i}", tag=f"whi{i}")
                  for i in range(2)]
        wlo_sb = [singles.tile([P, 3, 2, D], F8, name=f"wlo{i}", tag=f"wlo{i}")
                  for i in range(2)]
        wq_sb = [singles.tile([P, D], BF16, name=f"wq{i}", tag=f"wq{i}") for i in range(NIC)]
        wk_sb = [singles.tile([P, D], BF16, name=f"wk{i}", tag=f"wk{i}") for i in range(NIC)]
        wv_sb = [singles.tile([P, D], BF16, name=f"wv{i}", tag=f"wv{i}") for i in range(NIC)]
        wo_sb = [singles.tile([P, D], BF16, name=f"wo{i}", tag=f"wo{i}") for i in range(NIC)]
        trim_sb = singles.tile([P, 2, P], BF16, name="trim", tag="trim")
        eps_sb = singles.tile([P, 1], F32, name="eps", tag="eps")
        nc.vector.memset(eps_sb, EPS)
        tiny_sb = singles.tile([P, 1], F32, name="tiny", tag="tiny")
        nc.vector.memset(tiny_sb, TINY)
        zero_sb = singles.tile([P, 1], F32, name="zero", tag="zero")
        nc.vector.memset(zero_sb, 0.0)
        # bias vector for the ctx+denominator staging copy: 0 on the 64 ctx
        # partitions, TINY on the denominator row (partition 64)
        tiny64 = singles.tile([65, 1], F32, name="tiny64", tag="tiny64")
        nc.vector.memset(tiny64[0:64, :], 0.0)
        nc.vector.memset(tiny64[64:65, :], TINY)

        def load_xt(b, lead=False):
            xh = xt_pool.tile([P, NIC, SP], F8, name="xh", tag="xh")
            xl = xt_pool.tile([P, NIC, SP], F8, name="xl", tag="xl")
            nc.sync.dma_start(out=xh, in_=xhi[b])
            (nc.scalar if lead else nc.sync).dma_start(out=xl, in_=xlo[b])
            return (xh, xl)

        xt_cur = load_xt(0, lead=True)
        nc.sync.dma_start(out=whi_sb[0], in_=whi[:, 0:3])
        nc.scalar.dma_start(out=wlo_sb[0], in_=wlo[:, 0:3])
        nc.sync.dma_start(out=whi_sb[1], in_=whi[:, 3:6])
        nc.scalar.dma_start(out=wlo_sb[1], in_=wlo[:, 3:6])

        def load_proj_weights():
            # issued after front(0): needed only by mid(0)
            nc.gpsimd.dma_start(out=trim_sb, in_=trim[:])
            for i in range(NIC):
                nc.gpsimd.dma_start(out=wq_sb[i], in_=wq[i])
                nc.gpsimd.dma_start(out=wk_sb[i], in_=wk[i])
            for i in range(NIC):
                nc.scalar.dma_start(out=wv_sb[i], in_=wv[i])
                nc.gpsimd.dma_start(out=wo_sb[i], in_=wo[i])

        def conv_lhs(xt_sb, st, g):
            # x-window AP for DR group g=(k, cp): partitions = chan-in-chunk,
            # j in {0,1} selects chunk 2cp+j, free = 128 s-window cols.
            k, cp = g // 2, g % 2
            base = (2 * cp) * SP + st * P + k
            t = xt_sb[:, 0, 0]
            return bass.AP(
                tensor=t.tensor,
                offset=t.offset + base,
                ap=[[t.ap[0][0], P], [SP, 2], [1, P]],
            )

        def front(b, xt_pair):
            """conv (fp8 DR, 3 hi/lo terms) + LN + batched h-transpose."""
            xh, xl = xt_pair
            mv = stat_pool.tile([P, NST, 2], F32, name="mv", tag="mv")
            lnv = stat_pool.tile([P, NST], F32, name="lnv", tag="lnv")
            rstd = stat_pool.tile([P, NST], F32, name="rstd", tag="rstd")
            htall = ht_pool.tile([P, NIC, S], BF16, name="htall", tag="htall")
            a_list = []
            terms = ((xh, whi_sb), (xh, wlo_sb), (xl, whi_sb))
            for st in range(NST):
                aps = ps_a.tile([P, D], F32, name="aps", tag="aps")
                n = 0
                for xt_sb, w_sb in terms:
                    for g in range(NCG):
                        nc.tensor.matmul(
                            aps,
                            lhsT=conv_lhs(xt_sb, st, g),
                            rhs=w_sb[g // 3][:, g % 3, :, :],
                            start=(n == 0),
                            stop=(n == 3 * NCG - 1),
                            perf_mode=DRM,
                        )
                        n += 1
                asb = a_pool.tile([P, D], F32, name="asb", tag="asb")
                nc.scalar.copy(asb, aps)
                stats = stat_pool.tile([P, 6], F32, name="bnst", tag="bnst")
                nc.vector.bn_stats(out=stats, in_=asb)
                nc.vector.bn_aggr(out=mv[:, st, :], in_=stats)
                a_list.append(asb)
            nc.scalar.activation(lnv, mv[:, :, 1], AF.Ln, bias=eps_sb, scale=1.0)
            nc.scalar.activation(rstd, lnv, AF.Exp, bias=zero_sb, scale=-0.5)
            for st in range(NST):
                hsb = h_pool.tile([P, D], BF16, name="hsb", tag="hsb")
                nc.vector.tensor_scalar(
                    hsb,
                    a_list[st],
                    scalar1=mv[:, st, 0:1],
                    scalar2=rstd[:, st : st + 1],
                    op0=mybir.AluOpType.subtract,
                    op1=mybir.AluOpType.mult,
                )
                nc.sync.dma_start(
                    out=htall[:, :, st * P : (st + 1) * P], in_=hsb, transpose=True
                )
            return htall

        def mid(b, htall, prev_tail, last=False):
            """projections + attention + per-pair normalize for batch b;
            out-proj s-tiles of prev_tail=(b-2, cpack, oall) are interleaved
            at head-pair boundaries so PE stays busy while ACT drains the
            pair-boundary copies."""
            qt_sb = []
            kt_sb = []
            for oc in range(NIC):
                qps = ps_mm.tile([P, S], F32, name="qps", tag="qps")
                for i in range(NIC):
                    nc.tensor.matmul(
                        qps,
                        lhsT=wq_sb[i][:, oc * P : (oc + 1) * P],
                        rhs=htall[:, i, :],
                        start=(i == 0),
                        stop=(i == NIC - 1),
                    )
                qsb = qk_pool.tile([P, S], BF16, name="qtsb", tag="qtsb")
                nc.vector.tensor_copy(qsb, qps)
                qt_sb.append(qsb)

                kps = ps_mm.tile([P, S], F32, name="qps", tag="qps")
                for i in range(NIC):
                    nc.tensor.matmul(
                        kps,
                        lhsT=wk_sb[i][:, oc * P : (oc + 1) * P],
                        rhs=htall[:, i, :],
                        start=(i == 0),
                        stop=(i == NIC - 1),
                    )
                ksb = qk_pool.tile([P, S], BF16, name="qtsb", tag="qtsb")
                nc.vector.tensor_copy(ksb, kps)
                kt_sb.append(ksb)

            v_aug = []
            for st in range(NST):
                vps = ps_mm.tile([P, D], F32, name="qps", tag="qps")
                for i in range(NIC):
                    nc.tensor.matmul(
                        vps,
                        lhsT=htall[:, i, st * P : (st + 1) * P],
                        rhs=wv_sb[i],
                        start=(i == 0),
                        stop=(i == NIC - 1),
                    )
                vsb = v_pool.tile([P, H, 66], BF16, name="vsb", tag="vsb")
                nc.vector.memset(vsb[:, :, 64:66], 1.0)
                nc.scalar.copy(
                    vsb[:, :, 0:64], vps.rearrange("p (h d) -> p h d", h=H)
                )
                v_aug.append(vsb)

            cpack = cp_pool.tile([P, NHP, S], BF16, name="cpack", tag="cpack")
            ctmp = cp_pool.tile([DK, NHP, S], BF16, name="ctmp", tag="ctmp")

            def tail_mm_st(st):
                if prev_tail is None:
                    return
                tb, tcpack, toall = prev_tail
                ops = ps_mm.tile([P, D], F32, name="qps", tag="qps")
                for hp in range(NHP):
                    nc.tensor.matmul(
                        ops,
                        lhsT=tcpack[:, hp, st * P : (st + 1) * P],
                        rhs=wo_sb[hp],
                        start=(hp == 0),
                        stop=(hp == NHP - 1),
                    )
                if st % 2 == 0:
                    nc.scalar.copy(toall[:, st, :], ops)
                else:
                    nc.vector.tensor_copy(toall[:, st, :], ops)
                if st == NST - 1:
                    nc.sync.dma_start(
                        out=out[tb].rearrange("(st p) d -> p st d", st=NST),
                        in_=toall,
                    )

            def make_trim(pt, ki):
                tsl = trim_sb[:, 1 if ki == 0 else 0, :]
                tbc = bass.AP(
                    tensor=tsl.tensor,
                    offset=tsl.offset,
                    ap=[tsl.ap[0], [0, 2], [1, P]],
                )
                nc.vector.tensor_mul(pt[:, :, 0:P], pt[:, :, 0:P], tbc)

            for hp in range(NHP):
                cps2 = [
                    ps_cx.tile([65, S], F32, name="cps", tag="cps") for _ in range(2)
                ]
                for ki in range(NST):
                    qoff = ki * P
                    nq = S - qoff
                    sps = ps_sc.tile([P, 2, S], F32, name="sps", tag="sps")
                    for e in range(2):
                        hr = e * DK
                        nc.tensor.matmul(
                            sps[:, e, 0:nq],
                            lhsT=kt_sb[hp][hr : hr + DK, ki * P : (ki + 1) * P],
                            rhs=qt_sb[hp][hr : hr + DK, qoff:S],
                            start=True,
                            stop=True,
                        )
                    pt = pt_pool.tile([P, 2, S], BF16, name="pt", tag="pt")
                    nc.scalar.activation(
                        pt[:, :, 0:nq], sps[:, :, 0:nq], AF.Exp, scale=0.125
                    )
                    make_trim(pt, ki)
                    for e in range(2):
                        nc.tensor.matmul(
                            cps2[e][:, qoff:S],
                            lhsT=v_aug[ki][:, 2 * hp + e, 0:65],
                            rhs=pt[:, e, 0:nq],
                            start=(ki == 0),
                            stop=(ki == NST - 1),
                        )
                # pair boundary: stage ctx+denominator rows off PSUM on DVE
                # (+TINY only on the denominator partition), interleave the
                # out-proj s-tile of b-2, then denominator chain.
                cxp = cx_pool.tile([65, 2, S], F32, name="cxp", tag="cxp")
                for e in range(2):
                    nc.scalar.activation(
                        cxp[:, e, :], cps2[e], AF.Identity,
                        bias=tiny64, scale=1.0,
                    )
                tail_mm_st(hp)
                rall = r_pool.tile([DK, 2, S], F32, name="rall", tag="rall")
                rsrc = cxp[64:65, 0, 0]
                rsrc = bass.AP(
                    tensor=rsrc.tensor,
                    offset=rsrc.offset,
                    ap=[[rsrc.ap[0][0], 1], [0, DK], [S, 2], [1, S]],
                )
                nc.sync.dma_start(out=rall, in_=rsrc)
                nc.vector.reciprocal_approx_fast(out=rall, in_=rall)
                nc.vector.tensor_mul(cpack[0:DK, hp, :], cxp[0:DK, 0, :], rall[:, 0, :])
                nc.vector.tensor_mul(ctmp[:, hp, :], cxp[0:DK, 1, :], rall[:, 1, :])
            nc.sync.dma_start(out=cpack[DK:P, :, :], in_=ctmp)
            oall = o_pool.tile([P, NST, D], F32, name="oall", tag="oall")
            return (b, cpack, oall)

        def tail_mm_solo(prev_tail):
            tb, tcpack, toall = prev_tail
            for st in range(NST):
                ops = ps_mm.tile([P, D], F32, name="qps", tag="qps")
                for hp in range(NHP):
                    nc.tensor.matmul(
                        ops,
                        lhsT=tcpack[:, hp, st * P : (st + 1) * P],
                        rhs=wo_sb[hp],
                        start=(hp == 0),
                        stop=(hp == NHP - 1),
                    )
                if st % 2 == 0:
                    nc.scalar.copy(toall[:, st, :], ops)
                else:
                    nc.vector.tensor_copy(toall[:, st, :], ops)
                nc.sync.dma_start(
                    out=out[tb, st * P : (st + 1) * P, :], in_=toall[:, st, :]
                )

        # 3-stage pipeline: [front(b) | mid(b-1) w/ interleaved out-proj(b-2)]
        pend_mid = None
        pend_tail = None
        for b in range(BL):
            xt_next = load_xt(b + 1) if b + 1 < BL else None
            ht = front(b, xt_cur)
            if b == 0:
                load_proj_weights()
            if pend_mid is not None:
                pend_tail = mid(*pend_mid, pend_tail)
            pend_mid = (b, ht)
            xt_cur = xt_next
        pend_tail = mid(*pend_mid, pend_tail, last=True)
        tail_mm_solo(pend_tail)

    nc.compile()
    return nc


def _q8(a):
    """fp8e4 round-to-nearest via ml_dtypes."""
    return a.astype(ml_dtypes.float8_e4m3)


def prep_inputs(inputs):
    """Host-side prep: shard over batch, fold scales into weights, fp8 hi/lo
    splits of x and conv weights, pre-transpose x."""
    x = np.asarray(inputs["x"], np.float32)
    conv_w = np.asarray(inputs["conv_w"], np.float32)
    conv_b = np.asarray(inputs["conv_b"], np.float32)
    sb = np.asarray(inputs["sqrt_beta"], np.float32).reshape(D)
    ln_w = np.asarray(inputs["ln_w"], np.float32)
    ln_b = np.asarray(inputs["ln_b"], np.float32)
    Wq = np.asarray(inputs["Wq"], np.float32)
    Wk = np.asarray(inputs["Wk"], np.float32)
    Wv = np.asarray(inputs["Wv"], np.float32)
    Wo = np.asarray(inputs["Wo"], np.float32)
    mask = np.asarray(inputs["mask"])

    for nm in ("bq", "bk", "bv", "bo"):
        assert not np.any(np.asarray(inputs[nm])), f"{nm} must be zero"
    assert not np.any(conv_b), "conv_b must be zero"
    assert not np.any(ln_b), "ln_b must be zero"
    assert np.array_equal(
        mask.reshape(S, S), np.tril(np.ones((S, S), mask.dtype))
    ), "mask must be causal"

    c1 = 1.0 - sb * sb
    c2 = 1.0 + sb * sb
    Wp = conv_w * c1[:, None, None]  # [o, i, k]
    Wp[np.arange(D), np.arange(D), 2] += c2

    # fp8 scales (power of 2, amax -> ~240)
    sW = 2.0 ** np.floor(np.log2(240.0 / np.abs(Wp).max()))
    sX = 2.0 ** np.floor(np.log2(240.0 / np.abs(x).max()))

    # weights: w[p, (k, cp), j, o] = Wp[o, (2cp+j)*128+p, k] * sW, hi + lo
    Ws = Wp * sW  # [o, i, k]
    wt = Ws.transpose(1, 2, 0).reshape(NIC, P, KW, D)  # [i-chunk, p, k, o]
    warr = np.empty((P, NCG, 2, D), np.float32)
    for k in range(KW):
        for cp in range(2):
            for j in range(2):
                warr[:, k * 2 + cp, j, :] = wt[2 * cp + j, :, k, :]
    whi = _q8(warr)
    wlo = _q8(warr - whi.astype(np.float32))

    def fold(W):  # [o, i] -> [ic, il, o] with ln_w folded on i
        Wf = W * ln_w[None, :]
        return np.ascontiguousarray(Wf.T).reshape(NIC, P, D)

    wq_h, wk_h, wv_h = fold(Wq), fold(Wk), fold(Wv)
    wo_h = np.ascontiguousarray(Wo.T).reshape(NIC, P, D)

    tri = np.triu(np.ones((P, P), np.float32))
    tri0 = tri.copy()
    tri0[:, 0] = 0.0
    trim = np.stack([tri, tri0], axis=1)  # [P, 2, P]

    bf = ml_dtypes.bfloat16
    consts = {
        "whi": whi,
        "wlo": wlo,
        "wq": wq_h.astype(bf),
        "wk": wk_h.astype(bf),
        "wv": wv_h.astype(bf),
        "wo": wo_h.astype(bf),
        "trim": trim.astype(bf),
    }

    in_maps = []
    for c in range(NCORES):
        xs = x[c * BL : (c + 1) * BL] * sX  # [BL, S, D]
        xt = np.zeros((BL, D, SP), np.float32)
        xt[:, :, 2 : 2 + S] = xs.transpose(0, 2, 1)
        xh = _q8(xt)
        xl = _q8(xt - xh.astype(np.float32))
        m = dict(consts)
        m["xhi"] = np.ascontiguousarray(
            xh.reshape(BL, NIC, P, SP).transpose(0, 2, 1, 3)
        )
        m["xlo"] = np.ascontiguousarray(
            xl.reshape(BL, NIC, P, SP).transpose(0, 2, 1, 3)
        )
        in_maps.append(m)
    return in_maps


_NC_CACHE = {}


def get_nc():
    if "nc" not in _NC_CACHE:
        _NC_CACHE["nc"] = build_nc()
    return _NC_CACHE["nc"]


def kernel(**inputs):
    nc = get_nc()
    in_maps = prep_inputs(inputs)
    res = run_bass_kernel_spmd(nc, in_maps, list(range(NCORES)))
    outs = [np.asarray(r["out"], np.float32) for r in res.results]
    return np.concatenate(outs, axis=0)


if __name__ == "__main__":
    nc = build_nc()
    print("built ok")


# revision 39
# speedup vs baseline: 1.0616x; 1.0027x over previous
"""Trainium2 Bass kernel for nn_BAKTTime: causal-conv frequency layer + LN + causal MHA.

Sharding: pure data-parallel over batch - 8 of the 64 batch items per NeuronCore,
no collectives. Each core runs a 3-stage software-pipelined program
[conv+LN(b) | qkv+attention+normalize(b-1) with the out-projection of b-2
interleaved at head-pair boundaries] over its 8 batch items
(S=512, D=512, H=8, DK=64).

Key structure:
  - conv runs as fp8e4 DoubleRow matmuls (contract 256/instr, 0.5 cy/row) with
    a hi+lo residual split of both x and the folded conv weights (3 of the 4
    cross terms; dropped lo*lo ~ 2^-8 relative, so conv accuracy ~ bf16).
    Splits are host-prepped; LN absorbs the fp8 scale factors. SP=640 keeps
    the DR k-tile stride a multiple of 128 (walrus s3_lw_dual_fp8 check).
    Conv stream cost drops 24576 -> 18432 PE rows per batch.
  - batched DMAs (HWDGE fixed cost is ~632ns/DMA): 1 xt-hi + 1 xt-lo load,
    4 batched h-transposes (3D-out xbar transpose -> htall[dc, i, s]),
    1 denominator broadcast per head pair, 1 odd-head shift, 1 output store
    per batch (the last batch stores per s-tile to shorten the drain).
  - denominator chain per head pair: ctx+denominator rows staged off PSUM on
    DVE (+TINY only on the denominator partition via a [65,1] bias vector),
    one [1,2,S]->[64,2,S] f32 broadcast DMA, reciprocal_approx_fast in place
    on the broadcasted tile (a partition-64-based DVE write lands on the
    wrong partitions, so never reciprocal the raw row), then the normalize
    muls into the head-pair tile cpack.
  - engine balance per batch (~ns, PE-bound): PE ~29900 (71680 rows),
    DVE ~22000 (bn stats, q/k PSUM drains, chains), ACT ~20000 (softmax exp,
    conv-PSUM staging, v drains, rstd, half the out-proj drains),
    Pool ~11000 (h-pass, probs trim muls). The out-proj s-tiles of b-2 are
    interleaved at pair boundaries so PE stays busy while the chains run.
"""

import sys

sys.path.insert(0, "/opt/trn_rl_repo")

import numpy as np
import ml_dtypes
from contextlib import ExitStack

import concourse.bass as bass
from concourse import bacc
import concourse.mybir as mybir
import concourse.tile as tile
from concourse.bass_utils import run_bass_kernel_spmd

# Force Exp and Ln to resolve to the single table set that contains both
# (natural_log_exp_and_others), so ACT doesn't thrash table loads.
import concourse.hw_specs as _hw_specs

_orig_get_tables = _hw_specs.get_activation_tables


def _patched_get_tables(arch):
    t = dict(_orig_get_tables(arch))
    exp = mybir.ActivationFunctionType.Exp
    ln = mybir.ActivationFunctionType.Ln
    for name, funcs in t.items():
        if name != "natural_log_exp_and_others" and (exp in funcs or ln in funcs):
            t[name] = funcs - {exp, ln}
    return t


_hw_specs.get_activation_tables = _patched_get_tables
bacc.get_activation_tables = _patched_get_tables

B, S, D, H, KW = 64, 512, 512, 8, 3
DK = D // H  # 64
NCORES = 8
BL = B // NCORES  # 8 batches per core
P = 128
NST = S // P  # 4 s-tiles
NIC = D // P  # 4 input-chunks
NHP = H // 2  # 4 head pairs
SP = 640  # padded xT free dim: 2 zero cols + 512 data + 126 pad (mult of 128)
EPS = 1e-12
TINY = 2e-5  # q=0 denominator guard; 1/TINY = 5e4 < f16 max, and the
# +TINY bias the staging copy adds to ctx rows is negligible (~2e-5 abs)
F32 = mybir.dt.float32
F16 = mybir.dt.float16
BF16 = mybir.dt.bfloat16
F8 = mybir.dt.float8e4
DRM = mybir.MatmulPerfMode.DoubleRow
AF = mybir.ActivationFunctionType
NCG = 6  # conv DR groups: (tap k, chunk-pair cp)


def build_nc():
    nc = bacc.Bacc("TRN2", target_bir_lowering=False)
    xhi = nc.declare_dram_parameter("xhi", [BL, P, NIC, SP], F8, isOutput=False)
    xlo = nc.declare_dram_parameter("xlo", [BL, P, NIC, SP], F8, isOutput=False)
    whi = nc.declare_dram_parameter("whi", [P, NCG, 2, D], F8, isOutput=False)
    wlo = nc.declare_dram_parameter("wlo", [P, NCG, 2, D], F8, isOutput=False)
    wq = nc.declare_dram_parameter("wq", [NIC, P, D], BF16, isOutput=False)
    wk = nc.declare_dram_parameter("wk", [NIC, P, D], BF16, isOutput=False)
    wv = nc.declare_dram_parameter("wv", [NIC, P, D], BF16, isOutput=False)
    wo = nc.declare_dram_parameter("wo", [NIC, P, D], BF16, isOutput=False)
    trim = nc.declare_dram_parameter("trim", [P, 2, P], BF16, isOutput=False)
    out = nc.declare_dram_parameter("out", [BL, S, D], F32, isOutput=True)

    with ExitStack() as ctx:
        tc = ctx.enter_context(tile.TileContext(nc))
        singles = ctx.enter_context(tc.tile_pool(name="singles", bufs=1))
        xt_pool = ctx.enter_context(tc.tile_pool(name="xt", bufs=2))
        a_pool = ctx.enter_context(tc.tile_pool(name="a", bufs=6))
        stat_pool = ctx.enter_context(tc.tile_pool(name="stat", bufs=4))
        h_pool = ctx.enter_context(tc.tile_pool(name="h", bufs=8))
        ht_pool = ctx.enter_context(tc.tile_pool(name="ht", bufs=2))
        qk_pool = ctx.enter_context(tc.tile_pool(name="qk", bufs=16))
        v_pool = ctx.enter_context(tc.tile_pool(name="v", bufs=8))
        pt_pool = ctx.enter_context(tc.tile_pool(name="pt", bufs=6))
        dn_pool = ctx.enter_context(tc.tile_pool(name="dn", bufs=3))
        r_pool = ctx.enter_context(tc.tile_pool(name="r", bufs=3))
        cx_pool = ctx.enter_context(tc.tile_pool(name="cx", bufs=4))
        cp_pool = ctx.enter_context(tc.tile_pool(name="cp", bufs=3))
        o_pool = ctx.enter_context(tc.tile_pool(name="o", bufs=2))
        ps_a = ctx.enter_context(tc.tile_pool(name="ps_a", bufs=2, space="PSUM"))
        ps_mm = ctx.enter_context(tc.tile_pool(name="ps_mm", bufs=2, space="PSUM"))
        ps_sc = ctx.enter_context(tc.tile_pool(name="ps_sc", bufs=1, space="PSUM"))
        ps_cx = ctx.enter_context(tc.tile_pool(name="ps_cx", bufs=2, space="PSUM"))

        # --- load weights once; first-batch critical path (xh, xl, whi_a,
        # wlo_a) leads on the two HWDGE queues, the rest follows ---
        whi_sb = [singles.tile([P, 3, 2, D], F8, name=f"whi{i}", tag=f"whi{i}")
                  for i in range(2)]
        wlo_sb = [singles.tile([P, 3, 2, D], F8, name=f"wlo{i}", tag=f"wlo{i}")
                  for i in range(2)]
        wq_sb = [singles.tile([P, D], BF16, name=f"wq{i}", tag=f"wq{i}") for i in range(NIC)]
        wk_sb = [singles.tile([P, D], BF16, name=f"wk{i}", tag=f"wk{i}") for i in range(NIC)]
        wv_sb = [singles.tile([P, D], BF16, name=f"wv{i}", tag=f"wv{i}") for i in range(NIC)]
        wo_sb = [singles.tile([P, D], BF16, name=f"wo{i}", tag=f"wo{i}") for i in range(NIC)]
        trim_sb = singles.tile([P, 2, P], BF16, name="trim", tag="trim")
        eps_sb = singles.tile([P, 1], F32, name="eps", tag="eps")
        nc.vector.memset(eps_sb, EPS)
        tiny_sb = singles.tile([P, 1], F32, name="tiny", tag="tiny")
        nc.vector.memset(tiny_sb, TINY)
        zero_sb = singles.tile([P, 1], F32, name="zero", tag="zero")
        nc.vector.memset(zero_sb, 0.0)
        # bias vector for the ctx+denominator staging copy: 0 on the 64 ctx
        # partitions, TINY on the denominator row (partition 64)
        tiny64 = singles.tile([65, 1], F32, name="tiny64", tag="tiny64")
        nc.vector.memset(tiny64[0:64, :], 0.0)
        nc.vector.memset(tiny64[64:65, :], TINY)

        def load_xt(b, lead=False):
            xh = xt_pool.tile([P, NIC, SP], F8, name="xh", tag="xh")
            xl = xt_pool.tile([P, NIC, SP], F8, name="xl", tag="xl")
            nc.sync.dma_start(out=xh, in_=xhi[b])
            (nc.scalar if lead else nc.sync).dma_start(out=xl, in_=xlo[b])
            return (xh, xl)

        xt_cur = load_xt(0, lead=True)
        nc.sync.dma_start(out=whi_sb[0], in_=whi[:, 0:3])
        nc.scalar.dma_start(out=wlo_sb[0], in_=wlo[:, 0:3])
        nc.sync.dma_start(out=whi_sb[1], in_=whi[:, 3:6])
        nc.scalar.dma_start(out=wlo_sb[1], in_=wlo[:, 3:6])

        def load_proj_weights():
            # issued after front(0): needed only by mid(0)
            nc.gpsimd.dma_start(out=trim_sb, in_=trim[:])
            for i in range(NIC):
                nc.gpsimd.dma_start(out=wq_sb[i], in_=wq[i])
                nc.gpsimd.dma_start(out=wk_sb[i], in_=wk[i])
            for i in range(NIC):
                nc.scalar.dma_start(out=wv_sb[i], in_=wv[i])
                nc.gpsimd.dma_start(out=wo_sb[i], in_=wo[i])

        def conv_lhs(xt_sb, st, g):
            # x-window AP for DR group g=(k, cp): partitions = chan-in-chunk,
            # j in {0,1} selects chunk 2cp+j, free = 128 s-window cols.
            k, cp = g // 2, g % 2
            base = (2 * cp) * SP + st * P + k
            t = xt_sb[:, 0, 0]
            return bass.AP(
                tensor=t.tensor,
                offset=t.offset + base,
                ap=[[t.ap[0][0], P], [SP, 2], [1, P]],
            )

        def front(b, xt_pair):
            """conv (fp8 DR, 3 hi/lo terms) + LN + batched h-transpose."""
            xh, xl = xt_pair
            mv = stat_pool.tile([P, NST, 2], F32, name="mv", tag="mv")
            lnv = stat_pool.tile([P, NST], F32, name="lnv", tag="lnv")
            rstd = stat_pool.tile([P, NST], F32, name="rstd", tag="rstd")
            htall = ht_pool.tile([P, NIC, S], BF16, name="htall", tag="htall")
            a_list = []
            terms = ((xh, whi_sb), (xh, wlo_sb), (xl, whi_sb))
            for st in range(NST):
                aps = ps_a.tile([P, D], F32, name="aps", tag="aps")
                n = 0
                for xt_sb, w_sb in terms:
                    for g in range(NCG):
                        nc.tensor.matmul(
                            aps,
                            lhsT=conv_lhs(xt_sb, st, g),
                            rhs=w_sb[g // 3][:, g % 3, :, :],
                            start=(n == 0),
                            stop=(n == 3 * NCG - 1),
                            perf_mode=DRM,
                        )
                        n += 1
                asb = a_pool.tile([P, D], F32, name="asb", tag="asb")
                nc.scalar.copy(asb, aps)
                stats = stat_pool.tile([P, 6], F32, name="bnst", tag="bnst")
                nc.vector.bn_stats(out=stats, in_=asb)
                nc.vector.bn_aggr(out=mv[:, st, :], in_=stats)
                a_list.append(asb)
            nc.scalar.activation(lnv, mv[:, :, 1], AF.Ln, bias=eps_sb, scale=1.0)
            nc.scalar.activation(rstd, lnv, AF.Exp, bias=zero_sb, scale=-0.5)
            for st in range(NST):
                hsb = h_pool.tile([P, D], BF16, name="hsb", tag="hsb")
                nc.vector.tensor_scalar(
                    hsb,
                    a_list[st],
                    scalar1=mv[:, st, 0:1],
                    scalar2=rstd[:, st : st + 1],
                    op0=mybir.AluOpType.subtract,
                    op1=mybir.AluOpType.mult,
                )
                nc.sync.dma_start(
                    out=htall[:, :, st * P : (st + 1) * P], in_=hsb, transpose=True
                )
            return htall

        def mid(b, htall, prev_tail, last=False):
            """projections + attention + per-pair normalize for batch b;
            out-proj s-tiles of prev_tail=(b-2, cpack, oall) are interleaved
            at head-pair boundaries so PE stays busy while ACT drains the
            pair-boundary copies."""
            qt_sb = []
            kt_sb = []
            for oc in range(NIC):
                qps = ps_mm.tile([P, S], F32, name="qps", tag="qps")
                for i in range(NIC):
                    nc.tensor.matmul(
                        qps,
                        lhsT=wq_sb[i][:, oc * P : (oc + 1) * P],
                        rhs=htall[:, i, :],
                        start=(i == 0),
                        stop=(i == NIC - 1),
                    )
                qsb = qk_pool.tile([P, S], BF16, name="qtsb", tag="qtsb")
                nc.vector.tensor_copy(qsb, qps)
                qt_sb.append(qsb)

                kps = ps_mm.tile([P, S], F32, name="qps", tag="qps")
                for i in range(NIC):
                    nc.tensor.matmul(
                        kps,
                        lhsT=wk_sb[i][:, oc * P : (oc + 1) * P],
                        rhs=htall[:, i, :],
                        start=(i == 0),
                        stop=(i == NIC - 1),
                    )
                ksb = qk_pool.tile([P, S], BF16, name="qtsb", tag="qtsb")
                nc.vector.tensor_copy(ksb, kps)
                kt_sb.append(ksb)

            v_aug = []
            for st in range(NST):
                vps = ps_mm.tile([P, D], F32, name="qps", tag="qps")
                for i in range(NIC):
                    nc.tensor.matmul(
                        vps,
                        lhsT=htall[:, i, st * P : (st + 1) * P],
                        rhs=wv_sb[i],
                        start=(i == 0),
                        stop=(i == NIC - 1),
                    )
                vsb = v_pool.tile([P, H, 66], BF16, name="vsb", tag="vsb")
                nc.vector.memset(vsb[:, :, 64:66], 1.0)
                nc.scalar.copy(
                    vsb[:, :, 0:64], vps.rearrange("p (h d) -> p h d", h=H)
                )
                v_aug.append(vsb)

            cpack = cp_pool.tile([P, NHP, S], BF16, name="cpack", tag="cpack")
            ctmp = cp_pool.tile([DK, NHP, S], BF16, name="ctmp", tag="ctmp")

            def tail_mm_st(st):
                if prev_tail is None:
                    return
                tb, tcpack, toall = prev_tail
                ops = ps_mm.tile([P, D], F32, name="qps", tag="qps")
                for hp in range(NHP):
                    nc.tensor.matmul(
                        ops,
                        lhsT=tcpack[:, hp, st * P : (st + 1) * P],
                        rhs=wo_sb[hp],
                        start=(hp == 0),
                        stop=(hp == NHP - 1),
                    )
                if st % 2 == 0:
                    nc.scalar.copy(toall[:, st, :], ops)
                else:
                    nc.vector.tensor_copy(toall[:, st, :], ops)
                if st == NST - 1:
                    nc.sync.dma_start(
                        out=out[tb].rearrange("(st p) d -> p st d", st=NST),
                        in_=toall,
                    )

            def make_trim(pt, ki):
                tsl = trim_sb[:, 1 if ki == 0 else 0, :]
                tbc = bass.AP(
                    tensor=tsl.tensor,
                    offset=tsl.offset,
                    ap=[tsl.ap[0], [0, 2], [1, P]],
                )
                nc.vector.tensor_mul(pt[:, :, 0:P], pt[:, :, 0:P], tbc)

            for hp in range(NHP):
                cps2 = [
                    ps_cx.tile([65, S], F32, name="cps", tag="cps") for _ in range(2)
                ]
                for ki in range(NST):
                    qoff = ki * P
                    nq = S - qoff
                    sps = ps_sc.tile([P, 2, S], F32, name="sps", tag="sps")
                    for e in range(2):
                        hr = e * DK
                        nc.tensor.matmul(
                            sps[:, e, 0:nq],
                            lhsT=kt_sb[hp][hr : hr + DK, ki * P : (ki + 1) * P],
                            rhs=qt_sb[hp][hr : hr + DK, qoff:S],
                            start=True,
                            stop=True,
                        )
                    pt = pt_pool.tile([P, 2, S], BF16, name="pt", tag="pt")
                    nc.scalar.activation(
                        pt[:, :, 0:nq], sps[:, :, 0:nq], AF.Exp, scale=0.125
                    )
                    make_trim(pt, ki)
                    for e in range(2):
                        nc.tensor.matmul(
                            cps2[e][:, qoff:S],
                            lhsT=v_aug[ki][:, 2 * hp + e, 0:65],
                            rhs=pt[:, e, 0:nq],
                            start=(ki == 0),
                            stop=(ki == NST - 1),
                        )
                # pair boundary: stage ctx+denominator rows off PSUM on DVE
                # (+TINY only on the denominator partition), interleave the
                # out-proj s-tile of b-2, then denominator chain.
                cxp = cx_pool.tile([65, 2, S], F32, name="cxp", tag="cxp")
                for e in range(2):
                    nc.scalar.activation(
                        cxp[:, e, :], cps2[e], AF.Identity,
                        bias=tiny64, scale=1.0,
                    )
                tail_mm_st(hp)
                rall = r_pool.tile([DK, 2, S], F32, name="rall", tag="rall")
                rsrc = cxp[64:65, 0, 0]
                rsrc = bass.AP(
                    tensor=rsrc.tensor,
                    offset=rsrc.offset,
                    ap=[[rsrc.ap[0][0], 1], [0, DK], [S, 2], [1, S]],
                )
                nc.sync.dma_start(out=rall, in_=rsrc)
                nc.vector.reciprocal_approx_fast(out=rall, in_=rall)
                nc.vector.tensor_mul(cpack[0:DK, hp, :], cxp[0:DK, 0, :], rall[:, 0, :])
                nc.gpsimd.tensor_mul(ctmp[:, hp, :], cxp[0:DK, 1, :], rall[:, 1, :])
            nc.sync.dma_start(out=cpack[DK:P, :, :], in_=ctmp)
            oall = o_pool.tile([P, NST, D], F32, name="oall", tag="oall")
            return (b, cpack, oall)

        def tail_mm_solo(prev_tail):
            tb, tcpack, toall = prev_tail
            for st in range(NST):
                ops = ps_mm.tile([P, D], F32, name="qps", tag="qps")
                for hp in range(NHP):
                    nc.tensor.matmul(
                        ops,
                        lhsT=tcpack[:, hp, st * P : (st + 1) * P],
                        rhs=wo_sb[hp],
                        start=(hp == 0),
                        stop=(hp == NHP - 1),
                    )
                if st % 2 == 0:
                    nc.scalar.copy(toall[:, st, :], ops)
                else:
                    nc.vector.tensor_copy(toall[:, st, :], ops)
                nc.sync.dma_start(
                    out=out[tb, st * P : (st + 1) * P, :], in_=toall[:, st, :]
                )

        # 3-stage pipeline: [front(b) | mid(b-1) w/ interleaved out-proj(b-2)]
        pend_mid = None
        pend_tail = None
        for b in range(BL):
            xt_next = load_xt(b + 1) if b + 1 < BL else None
            ht = front(b, xt_cur)
            if b == 0:
                load_proj_weights()
            if pend_mid is not None:
                pend_tail = mid(*pend_mid, pend_tail)
            pend_mid = (b, ht)
            xt_cur = xt_next
        pend_tail = mid(*pend_mid, pend_tail, last=True)
        tail_mm_solo(pend_tail)

    nc.compile()
    return nc


def _q8(a):
    """fp8e4 round-to-nearest via ml_dtypes."""
    return a.astype(ml_dtypes.float8_e4m3)


def prep_inputs(inputs):
    """Host-side prep: shard over batch, fold scales into weights, fp8 hi/lo
    splits of x and conv weights, pre-transpose x."""
    x = np.asarray(inputs["x"], np.float32)
    conv_w = np.asarray(inputs["conv_w"], np.float32)
    conv_b = np.asarray(inputs["conv_b"], np.float32)
    sb = np.asarray(inputs["sqrt_beta"], np.float32).reshape(D)
    ln_w = np.asarray(inputs["ln_w"], np.float32)
    ln_b = np.asarray(inputs["ln_b"], np.float32)
    Wq = np.asarray(inputs["Wq"], np.float32)
    Wk = np.asarray(inputs["Wk"], np.float32)
    Wv = np.asarray(inputs["Wv"], np.float32)
    Wo = np.asarray(inputs["Wo"], np.float32)
    mask = np.asarray(inputs["mask"])

    for nm in ("bq", "bk", "bv", "bo"):
        assert not np.any(np.asarray(inputs[nm])), f"{nm} must be zero"
    assert not np.any(conv_b), "conv_b must be zero"
    assert not np.any(ln_b), "ln_b must be zero"
    assert np.array_equal(
        mask.reshape(S, S), np.tril(np.ones((S, S), mask.dtype))
    ), "mask must be causal"

    c1 = 1.0 - sb * sb
    c2 = 1.0 + sb * sb
    Wp = conv_w * c1[:, None, None]  # [o, i, k]
    Wp[np.arange(D), np.arange(D), 2] += c2

    # fp8 scales (power of 2, amax -> ~240)
    sW = 2.0 ** np.floor(np.log2(240.0 / np.abs(Wp).max()))
    sX = 2.0 ** np.floor(np.log2(240.0 / np.abs(x).max()))

    # weights: w[p, (k, cp), j, o] = Wp[o, (2cp+j)*128+p, k] * sW, hi + lo
    Ws = Wp * sW  # [o, i, k]
    wt = Ws.transpose(1, 2, 0).reshape(NIC, P, KW, D)  # [i-chunk, p, k, o]
    warr = np.empty((P, NCG, 2, D), np.float32)
    for k in range(KW):
        for cp in range(2):
            for j in range(2):
                warr[:, k * 2 + cp, j, :] = wt[2 * cp + j, :, k, :]
    whi = _q8(warr)
    wlo = _q8(warr - whi.astype(np.float32))

    def fold(W):  # [o, i] -> [ic, il, o] with ln_w folded on i
        Wf = W * ln_w[None, :]
        return np.ascontiguousarray(Wf.T).reshape(NIC, P, D)

    wq_h, wk_h, wv_h = fold(Wq), fold(Wk), fold(Wv)
    wo_h = np.ascontiguousarray(Wo.T).reshape(NIC, P, D)

    tri = np.triu(np.ones((P, P), np.float32))
    tri0 = tri.copy()
    tri0[:, 0] = 0.0
    trim = np.stack([tri, tri0], axis=1)  # [P, 2, P]

    bf = ml_dtypes.bfloat16
    consts = {
        "whi": whi,
        "wlo": wlo,
        "wq": wq_h.astype(bf),
        "wk": wk_h.astype(bf),
        "wv": wv_h.astype(bf),
        "wo": wo_h.astype(bf),
        "trim": trim.astype(bf),
    }

    in_maps = []
    for c in range(NCORES):
        xs = x[c * BL : (c + 1) * BL] * sX  # [BL, S, D]
        xt = np.zeros((BL, D, SP), np.float32)
        xt[:, :, 2 : 2 + S] = xs.transpose(0, 2, 1)
        xh = _q8(xt)
        xl = _q8(xt - xh.astype(np.float32))
        m = dict(consts)
        m["xhi"] = np.ascontiguousarray(
            xh.reshape(BL, NIC, P, SP).transpose(0, 2, 1, 3)
        )
        m["xlo"] = np.ascontiguousarray(
            xl.reshape(BL, NIC, P, SP).transpose(0, 2, 1, 3)
        )
        in_maps.append(m)
    return in_maps


_NC_CACHE = {}


def get_nc():
    if "nc" not in _NC_CACHE:
        _NC_CACHE["nc"] = build_nc()
    return _NC_CACHE["nc"]


def kernel(**inputs):
    nc = get_nc()
    in_maps = prep_inputs(inputs)
    res = run_bass_kernel_spmd(nc, in_maps, list(range(NCORES)))
    outs = [np.asarray(r["out"], np.float32) for r in res.results]
    return np.concatenate(outs, axis=0)


if __name__ == "__main__":
    nc = build_nc()
    print("built ok")
